# revision 1
# baseline (speedup 1.0000x reference)
"""Trainium2 Bass kernel for a transformer encoder layer (nn_Encoder).

x:[2,2048,1024] f32. 8 NeuronCores, data-parallel: core c handles batch
n=c//4, query rows qi=c%4 (512 tokens). K/V are recomputed per core for
the full batch (x4 redundancy) to avoid collectives, which are far too
slow (~300us for the 8.4MB all-reduce this would replace).
All matmuls run as float32r (full PE rate, ~1e-4 rel err).
"""
import os
import sys
from contextlib import ExitStack

for _p in ("/opt/trn_rl_repo", "/root/.axon_site/_ro/trn_rl_repo"):
    if os.path.isdir(_p) and _p not in sys.path:
        sys.path.insert(0, _p)

import numpy as np
import concourse.bass as bass
import concourse.mybir as mybir
import concourse.tile as tile
from concourse import bacc
from concourse.bass_utils import run_bass_kernel_spmd
from concourse.masks import make_identity

F32 = mybir.dt.float32
F32R = mybir.dt.float32r
AF = mybir.ActivationFunctionType
ALU = mybir.AluOpType

D = 1024
H = 16
HD = 64
FF = 4096
L = 2048
NB = 2
P = 128
QT = 512          # query tokens per core
DC = D // P       # 8 chunks of the model dim
KT = L // P       # 16 key tiles
FC = FF // P      # 32 ff chunks
TT = QT // P      # 4 own-token tiles
NPAIR = H // 2    # 8 head pairs
EPS = 1e-5

_CACHED_NC = {}


def _build_nc(affine=True):
    nc = bacc.Bacc("TRN2", target_bir_lowering=False)

    din = {}

    def dparam(name, shape, dt=F32R):
        din[name] = nc.dram_tensor(name, shape, dt, kind="ExternalInput")
        return din[name]

    xT = dparam("xT", [D, L])              # x[n].T
    xTq = dparam("xTq", [D, QT])           # own-token columns of xT
    xq = dparam("xq", [QT, D], F32)        # own tokens, natural (residual)
    wq = dparam("wq", [NPAIR, DC, P, P])   # [pair, dc, dpart, cols]
    wk = dparam("wk", [NPAIR, DC, P, P])
    wv = dparam("wv", [2, DC, P, D // 2])  # [vcol-half, dc, dpart, 512]
    wo = dparam("wo", [DC, P, D])          # [hd-chunk, hd-part, ocols]
    w1 = dparam("w1", [FC, DC, P, P])      # [fc, dc, dpart, fcols]
    w2 = dparam("w2", [FC, P, D])          # [fc, ff-part, ocols]
    bq = dparam("bq", [P, NPAIR], F32)     # per-partition bias, by pair
    bk = dparam("bk", [P, NPAIR], F32)
    b1 = dparam("b1", [P, FC], F32)
    bvb = dparam("bvb", [P, D], F32)       # host-broadcast per-column params
    b2b = dparam("b2b", [P, D], F32)
    g1b = dparam("g1b", [P, D], F32)
    be1b = dparam("be1b", [P, D], F32)
    g2b = dparam("g2b", [P, D], F32)
    be2b = dparam("be2b", [P, D], F32)
    vones = dparam("vones", [P, KT], F32R)

    y = nc.dram_tensor("y", [QT, D], F32, kind="ExternalOutput")

    with tile.TileContext(nc) as tc:
        with tc.tile_pool(name="pers", bufs=1) as pers:
            ident = pers.tile([P, P], F32, tag="ident")
            make_identity(nc, ident[:])
            bq_t = pers.tile([P, NPAIR], F32, tag="bq")
            bk_t = pers.tile([P, NPAIR], F32, tag="bk")
            b1_t = pers.tile([P, FC], F32, tag="b1")
            eps_t = pers.tile([P, 1], F32, tag="eps")
            nc.gpsimd.memset(eps_t[:], EPS)
            zero_t = pers.tile([P, 1], F32, tag="zero")
            nc.gpsimd.memset(zero_t[:], 0.0)

            # ---- persistent blobs (tag-shared slots across phases) ----
            # blobA: xT (proj) -> ff1T (ffn);  both 64KB/partition
            # blobB: v_aug (proj+attn) -> wo (out-proj) -> hT (ffn)
            # tok1:  xTq (q-proj) -> outSB (attn out, transposed)
            # tok2:  qT (proj+attn) -> h (post-LN1, natural)
            xT_t = pers.tile([P, DC, L], F32R, tag="blobA")
            _att_es = ExitStack()
            vap = _att_es.enter_context(tc.tile_pool(name="vap", bufs=1))
            v_aug = vap.tile([P, KT, H * (HD + 1)], F32R, tag="vaug")
            ones_t = pers.tile([P, KT], F32R, tag="ones")
            xTq_t = pers.tile([P, DC, QT], F32R, tag="tok1")
            nc.sync.dma_start(xTq_t[:], xTq.rearrange("(c p) t -> p c t", p=P))
            nc.scalar.dma_start(bq_t[:], bq[:])
            nc.scalar.dma_start(bk_t[:], bk[:])
            nc.scalar.dma_start(b1_t[:], b1[:])

            # ================= Q projection (own tokens) ==============
            qT_t = pers.tile([P, NPAIR, QT], F32R, tag="tok2")
            with tc.tile_pool(name="qp", bufs=2) as qp, \
                 tc.tile_pool(name="qps", bufs=4, space="PSUM") as qps:
                for pr2 in range(NPAIR // 2):
                    wq_t = qp.tile([P, 2, DC, P], F32R, tag="w")
                    nc.scalar.dma_start(
                        wq_t[:], wq[2 * pr2:2 * pr2 + 2].rearrange("r c p m -> p r c m"))
                    for r in range(2):
                        pr = 2 * pr2 + r
                        ps = qps.tile([P, 512], F32, tag="mm")
                        for dc in range(DC):
                            nc.tensor.matmul(ps[:], wq_t[:, r, dc, :], xTq_t[:, dc, :],
                                             start=(dc == 0), stop=(dc == DC - 1))
                        nc.vector.tensor_scalar(qT_t[:, pr, :], ps[:],
                                                bq_t[:, pr:pr + 1], None, ALU.add)

            for dc in range(DC):
                nc.sync.dma_start(
                    xT_t[:, dc, :],
                    xT.rearrange("(c p) t -> p c t", p=P)[:, dc, :])
            nc.scalar.dma_start(ones_t[:], vones[:])
            nc.vector.tensor_copy(
                v_aug[:].rearrange("p t (h c) -> p t h c", c=HD + 1)[:, :, :, HD],
                ones_t[:, :, None].to_broadcast([P, KT, H]))

            # ================= V projection (dc-outer, streamed wv) ===
            with tc.tile_pool(name="vp", bufs=1) as vp, \
                 tc.tile_pool(name="vpw", bufs=2) as vpw, \
                 tc.tile_pool(name="vps", bufs=1, space="PSUM") as vps:
                bvb_t = vp.tile([P, D], F32, tag="bvb")
                nc.scalar.dma_start(bvb_t[:], bvb[:])
                wv_ts = [vpw.tile([P, DC, 512], F32R, tag="wvh", name=f"wvh{vh}")
                         for vh in range(2)]
                # interleave wv[0] per-dc with the xT chunks so V's first
                # matmuls aren't stuck behind the full 8.4MB xT stream
                for dc in range(DC):
                    nc.scalar.dma_start(wv_ts[0][:, dc, :], wv[0, dc])
                nc.scalar.dma_start(wv_ts[1][:], wv[1].rearrange("c p m -> p c m"))
                for vh in range(2):
                    wv_t = wv_ts[vh]
                    for pas in range(4):
                        ps4 = [vps.tile([P, 512], F32, tag=f"vmm{i}",
                                        name=f"vps_{vh}_{pas}_{i}") for i in range(4)]
                        for dc in range(DC):
                            for i in range(4):
                                tt = pas * 4 + i
                                nc.tensor.matmul(
                                    ps4[i], xT_t[:, dc, tt * P:(tt + 1) * P],
                                    wv_t[:, dc, :], start=(dc == 0), stop=(dc == DC - 1))
                        for i in range(4):
                            tt = pas * 4 + i
                            dst = v_aug[:, tt, :].rearrange(
                                "p (h c) -> p h c", c=HD + 1)[:, vh * 8:(vh + 1) * 8, 0:HD]
                            nc.vector.tensor_tensor(
                                dst, ps4[i].rearrange("p (h c) -> p h c", c=HD),
                                bvb_t[:, vh * 512:(vh + 1) * 512].rearrange(
                                    "p (h c) -> p h c", c=HD),
                                ALU.add)

            # ========== K projection fused with attention, per pair ====
            # kT for a pair stays in SBUF (no DRAM bounce); PSUM budget:
            # K accum 2 + S^T 2x2 + PV accum 2 = 8 banks exactly.
            outSB = pers.tile([P, NPAIR, QT], F32R, tag="tok1")
            with tc.tile_pool(name="kp", bufs=2) as kp, \
                 tc.tile_pool(name="kq", bufs=2) as kq, \
                 tc.tile_pool(name="atp", bufs=3) as atp, \
                 tc.tile_pool(name="atd", bufs=2) as atd, \
                 tc.tile_pool(name="kps", bufs=1, space="PSUM") as kps, \
                 tc.tile_pool(name="stp", bufs=2, space="PSUM") as stpool, \
                 tc.tile_pool(name="pvp", bufs=2, space="PSUM") as pvpool:
                for pr in range(NPAIR):
                    wk_t = kp.tile([P, DC, P], F32R, tag="w")
                    eng = nc.scalar if pr % 2 else nc.sync
                    eng.dma_start(wk_t[:], wk[pr].rearrange("c p m -> p c m"))
                    kT_sb = kq.tile([P, L], F32R, tag="kts", name=f"kts_{pr}")
                    for half in range(2):
                        pst = [kps.tile([P, 512], F32, tag=f"kmm{i}",
                                        name=f"kps_{pr}_{half}_{i}") for i in range(2)]
                        for dc in range(DC):
                            for i in range(2):
                                t4 = half * 2 + i
                                nc.tensor.matmul(
                                    pst[i], wk_t[:, dc, :],
                                    xT_t[:, dc, t4 * 512:(t4 + 1) * 512],
                                    start=(dc == 0), stop=(dc == DC - 1))
                        for i in range(2):
                            t4 = half * 2 + i
                            nc.vector.tensor_scalar(
                                kT_sb[:, t4 * 512:(t4 + 1) * 512], pst[i],
                                bk_t[:, pr:pr + 1], None, ALU.add)
                    # ---- attention for this pair (kT_sb read in place) ----
                    pvs = [pvpool.tile([P, QT], F32, tag="pv", name=f"pv_{pr}_{h2}")
                           for h2 in range(2)]
                    for gi in range(KT // 2):
                        a = 2 * gi
                        for h2 in range(2):
                            h_idx = 2 * pr + h2
                            rows = slice(h2 * HD, h2 * HD + HD)
                            stp = stpool.tile([P, 1024], F32, tag="st",
                                              name=f"st_{pr}_{a}_{h2}")
                            for j in range(2):
                                kt = a + j
                                nc.tensor.matmul(
                                    stp[:, j * 512:(j + 1) * 512],
                                    kT_sb[rows, kt * P:(kt + 1) * P],
                                    qT_t[rows, pr, :], start=True, stop=True)
                            ptt = atp.tile([P, 2, QT], F32R, tag="pt",
                                           name=f"pt_{pr}_{a}_{h2}")
                            nc.scalar.activation(
                                ptt[:],
                                stp[:].rearrange("p (c n) -> p c n", n=512),
                                AF.Exp, scale=0.125)
                            for j in range(2):
                                kt = a + j
                                vsl = v_aug[:, kt, :].rearrange(
                                    "p (h c) -> p h c", c=HD + 1)[:, h_idx, :]
                                nc.tensor.matmul(pvs[h2][:HD + 1, :], vsl,
                                                 ptt[:, j, :],
                                                 start=(kt == 0), stop=(kt == KT - 1))
                    for h2 in range(2):
                        rows = slice(h2 * HD, h2 * HD + HD)
                        den = atd.tile([1, QT], F32, tag="den", name=f"den_{pr}_{h2}")
                        nc.vector.reciprocal(den[:], pvs[h2][HD:HD + 1, :])
                        denb = atd.tile([HD, QT], F32, tag="denb",
                                        name=f"denb_{pr}_{h2}")
                        nc.gpsimd.partition_broadcast(denb[:], den[:])
                        nc.vector.tensor_tensor(outSB[rows, pr, :], pvs[h2][:HD, :],
                                                denb[:], ALU.mult)

            _att_es.close()
            _ffn_es = ExitStack()
            fp = _ffn_es.enter_context(tc.tile_pool(name="fp", bufs=2))
            fw = _ffn_es.enter_context(tc.tile_pool(name="fw", bufs=1))

            # ================= Output proj + residual + LN1 ===========
            # Two tt-halves with 4 PSUM banks each: LN1 + hT transposes of
            # half A overlap half B's matmuls.
            h_t = pers.tile([P, TT, D], F32, tag="tok2")
            hT_t = pers.tile([P, DC, QT], F32R, tag="tok1")
            with tc.tile_pool(name="op", bufs=1) as op, \
                 tc.tile_pool(name="lnw", bufs=1) as lnw, \
                 tc.tile_pool(name="lnp3", bufs=2) as lnp3, \
                 tc.tile_pool(name="ops", bufs=1, space="PSUM") as ops, \
                 tc.tile_pool(name="tps", bufs=2, space="PSUM") as tps:
                if affine:
                    g1b_t = lnw.tile([P, D], F32, tag="g1b")
                    be1b_t = lnw.tile([P, D], F32, tag="be1b")
                    nc.scalar.dma_start(g1b_t[:], g1b[:])
                    nc.scalar.dma_start(be1b_t[:], be1b[:])
                else:
                    g1b_t = be1b_t = None
                wo_t = op.tile([P, DC, D], F32R, tag="wof")
                for pr in range(NPAIR):
                    eng = nc.scalar if pr % 2 else nc.sync
                    eng.dma_start(wo_t[:, pr, :], wo[pr])
                xq_s = op.tile([P, TT, D], F32, tag="xqs")
                nc.sync.dma_start(xq_s[:], xq.rearrange("(t p) d -> p t d", p=P))
                for half in range(2):
                    tts = (2 * half, 2 * half + 1)
                    pso = [ops.tile([P, 512], F32, tag=f"ao{i}",
                                    name=f"ao_{half}_{i}") for i in range(4)]
                    for pr in range(NPAIR):
                        for i, tt in enumerate(tts):
                            for oc in range(2):
                                nc.tensor.matmul(
                                    pso[i * 2 + oc],
                                    outSB[:, pr, tt * P:(tt + 1) * P],
                                    wo_t[:, pr, oc * 512:(oc + 1) * 512],
                                    start=(pr == 0), stop=(pr == NPAIR - 1))
                    for i, tt in enumerate(tts):
                        for oc in range(2):
                            nc.vector.tensor_tensor(
                                h_t[:, tt, oc * 512:(oc + 1) * 512],
                                pso[i * 2 + oc],
                                xq_s[:, tt, oc * 512:(oc + 1) * 512], ALU.add)
                        _layernorm(nc, lnp3, h_t[:, tt, :], h_t[:, tt, :],
                                   g1b_t, be1b_t, eps_t, zero_t, affine)
                        for dc in range(DC):
                            pst = tps.tile([P, P], F32, tag="tp",
                                           name=f"tp_{tt}_{dc}")
                            nc.tensor.transpose(
                                pst[:], h_t[:, tt, dc * P:(dc + 1) * P], ident[:])
                            nc.vector.tensor_copy(
                                hT_t[:, dc, tt * P:(tt + 1) * P], pst[:])

            # ================= FFN + LN2 ==============================
            with tc.tile_pool(name="ft", bufs=2) as ft, \
                 tc.tile_pool(name="lnp4", bufs=2) as lnp4:
                ff1 = pers.tile([P, FC, QT], F32R, tag="blobA")
                with tc.tile_pool(name="f1s", bufs=4, space="PSUM") as f1s:
                    for fc4 in range(FC // 4):
                        w1_t = fp.tile([P, 4, DC, P], F32R, tag="wstream")
                        eng = nc.scalar if fc4 % 2 else nc.sync
                        eng.dma_start(w1_t[:], w1[4 * fc4:4 * fc4 + 4].rearrange(
                            "f c p m -> p f c m"))
                        for f in range(4):
                            fc = 4 * fc4 + f
                            ps = f1s.tile([P, 512], F32, tag="mm")
                            for dc in range(DC):
                                nc.tensor.matmul(ps[:], w1_t[:, f, dc, :],
                                                 hT_t[:, dc, :],
                                                 start=(dc == 0), stop=(dc == DC - 1))
                            # fused bias + relu
                            nc.vector.tensor_scalar(ff1[:, fc, :], ps[:],
                                                    b1_t[:, fc:fc + 1], 0.0,
                                                    ALU.add, ALU.max)

                b2b_t = fw.tile([P, D], F32, tag="b2b")
                nc.scalar.dma_start(b2b_t[:], b2b[:])
                # fold the fc2 bias into the residual while ff1 runs
                for tt in range(TT):
                    nc.vector.tensor_tensor(h_t[:, tt, :], h_t[:, tt, :],
                                            b2b_t[:], ALU.add)
                if affine:
                    g2b_t = fw.tile([P, D], F32, tag="g2b")
                    be2b_t = fw.tile([P, D], F32, tag="be2b")
                    nc.scalar.dma_start(g2b_t[:], g2b[:])
                    nc.scalar.dma_start(be2b_t[:], be2b[:])
                else:
                    g2b_t = be2b_t = None
                with tc.tile_pool(name="f2s", bufs=1, space="PSUM") as f2s:
                    pss = [f2s.tile([P, 512], F32, tag=f"ff2_{i}", name=f"ff2_{i}") for i in range(8)]
                    for fc4 in range(FC // 4):
                        w2_t = fp.tile([P, 4, D], F32R, tag="wstream")
                        eng = nc.scalar if fc4 % 2 else nc.sync
                        eng.dma_start(w2_t[:], w2[4 * fc4:4 * fc4 + 4].rearrange(
                            "f p m -> p f m"))
                        for f in range(4):
                            fc = 4 * fc4 + f
                            for tt in range(TT):
                                for oc in range(2):
                                    nc.tensor.matmul(
                                        pss[tt * 2 + oc],
                                        ff1[:, fc, tt * P:(tt + 1) * P],
                                        w2_t[:, f, oc * 512:(oc + 1) * 512],
                                        start=(fc == 0), stop=(fc == FC - 1))
                    for tt in range(TT):
                        t2 = ft.tile([P, D], F32, tag="t2")
                        for oc in range(2):
                            nc.vector.tensor_tensor(
                                t2[:, oc * 512:(oc + 1) * 512],
                                pss[tt * 2 + oc],
                                h_t[:, tt, oc * 512:(oc + 1) * 512], ALU.add)
                        _layernorm(nc, lnp4, t2[:], t2[:], g2b_t, be2b_t, eps_t, zero_t, affine)
                        nc.sync.dma_start(
                            y.rearrange("(t p) d -> p t d", p=P)[:, tt, :], t2[:])
            _ffn_es.close()

    nc.compile()
    return nc


def _layernorm(nc, pool, dst, src, g_t, be_t, eps_t, zero_t, affine):
    """dst = (src - mean)/sqrt(var + eps) [* g + be], row-wise over 1024.

    var = E[x^2] - mu^2 (safe here: |mu| << rms). The mean-reduce (DVE) and
    square+sum (ACT, accum_out) run concurrently; one Newton step refines
    rsqrt. c doubles as the square scratch before holding (src - mu).
    """
    mu = pool.tile([P, 1], F32, tag="ln_mu")
    nc.vector.tensor_reduce(mu[:], src, mybir.AxisListType.X, ALU.add)
    nc.vector.tensor_scalar_mul(mu[:], mu[:], 1.0 / D)
    c = pool.tile([P, D], F32, tag="ln_c")
    ss = pool.tile([P, 1], F32, tag="ln_ss")
    nc.scalar.activation(c[:], src, AF.Square, accum_out=ss[:])
    vv = pool.tile([P, 1], F32, tag="ln_v")
    nc.vector.tensor_scalar(vv[:], ss[:], 1.0 / D, EPS, ALU.mult, ALU.add)
    m2 = pool.tile([P, 1], F32, tag="ln_m2")
    nc.vector.tensor_tensor(m2[:], mu[:], mu[:], ALU.mult)
    nc.vector.tensor_tensor(vv[:], vv[:], m2[:], ALU.subtract)
    s = pool.tile([P, 1], F32, tag="ln_s")
    nc.scalar.activation(s[:], vv[:], AF.Sqrt, bias=zero_t[:])
    r = pool.tile([P, 1], F32, tag="ln_r")
    nc.vector.reciprocal(r[:], s[:])
    # one Newton step: r <- r * (1.5 - 0.5 * vv * r^2)
    t = pool.tile([P, 1], F32, tag="ln_t")
    nc.vector.tensor_tensor(t[:], r[:], r[:], ALU.mult)
    nc.vector.tensor_tensor(t[:], t[:], vv[:], ALU.mult)
    nc.vector.tensor_scalar(t[:], t[:], -0.5, 1.5, ALU.mult, ALU.add)
    nc.vector.tensor_tensor(r[:], r[:], t[:], ALU.mult)
    nc.vector.tensor_scalar(c[:], src, mu[:], None, ALU.subtract)
    nc.vector.tensor_scalar(dst, c[:], r[:], None, ALU.mult)
    if affine:
        nc.vector.tensor_tensor(dst, dst, g_t[:], ALU.mult)
        nc.vector.tensor_tensor(dst, dst, be_t[:], ALU.add)


def make_in_maps(x, w_qkv, b_qkv, w_o, b_o, g1, be1, w1, b1, w2, b2, g2, be2):
    f = np.float32
    x = np.asarray(x, f)
    w_qkv = np.asarray(w_qkv, f)
    b_qkv = np.asarray(b_qkv, f)
    bc = lambda v: np.ascontiguousarray(
        np.broadcast_to(np.asarray(v, f).reshape(1, D), (P, D)))
    shared = {
        "wq": np.ascontiguousarray(
            w_qkv[:, :D].reshape(DC, P, NPAIR, P).transpose(2, 0, 1, 3)),
        "wk": np.ascontiguousarray(
            w_qkv[:, D:2 * D].reshape(DC, P, NPAIR, P).transpose(2, 0, 1, 3)),
        "wv": np.ascontiguousarray(
            w_qkv[:, 2 * D:].reshape(DC, P, 2, 512).transpose(2, 0, 1, 3)),
        "wo": np.ascontiguousarray(np.asarray(w_o, f).reshape(DC, P, D)),
        "w1": np.ascontiguousarray(
            np.asarray(w1, f).reshape(DC, P, FC, P).transpose(2, 0, 1, 3)),
        "w2": np.ascontiguousarray(np.asarray(w2, f).reshape(FC, P, D)),
        "bq": np.ascontiguousarray(b_qkv[:D].reshape(NPAIR, P).T),
        "bk": np.ascontiguousarray(b_qkv[D:2 * D].reshape(NPAIR, P).T),
        "b1": np.ascontiguousarray(np.asarray(b1, f).reshape(FC, P).T),
        "bvb": bc(b_qkv[2 * D:]), "b2b": bc(b2),
        "g1b": bc(g1), "be1b": bc(be1), "g2b": bc(g2), "be2b": bc(be2),
        "vones": np.ones((P, KT), f),
    }
    in_maps = []
    for c in range(8):
        n, qi = divmod(c, 4)
        xTn = np.ascontiguousarray(x[n].T)
        m = dict(shared)
        m["xT"] = xTn
        m["xTq"] = np.ascontiguousarray(xTn[:, qi * QT:(qi + 1) * QT])
        m["xq"] = np.ascontiguousarray(x[n, qi * QT:(qi + 1) * QT, :]
                                 + np.asarray(b_o, f).reshape(1, D))
        in_maps.append(m)
    return in_maps


def get_nc(affine=True):
    if affine not in _CACHED_NC:
        _CACHED_NC[affine] = _build_nc(affine)
    return _CACHED_NC[affine]


def kernel(**inputs):
    in_maps = make_in_maps(**inputs)
    affine = not (np.all(np.asarray(inputs["g1"]) == 1)
                  and np.all(np.asarray(inputs["be1"]) == 0)
                  and np.all(np.asarray(inputs["g2"]) == 1)
                  and np.all(np.asarray(inputs["be2"]) == 0))
    nc = get_nc(affine)
    # The axon-proxied NRT occasionally reports a transient
    # NRT_EXEC_UNIT_UNRECOVERABLE on a cold first dispatch; a plain retry
    # has always succeeded with bit-identical results, so recover inline.
    last_err = None
    for _ in range(3):
        try:
            res = run_bass_kernel_spmd(nc, in_maps, list(range(8))).results
            break
        except Exception as e:  # noqa: BLE001
            last_err = e
    else:
        raise last_err
    y = np.empty((NB, L, D), np.float32)
    for c in range(8):
        n, qi = divmod(c, 4)
        y[n, qi * QT:(qi + 1) * QT] = res[c]["y"]
    return y


if __name__ == "__main__":
    rng = np.random.default_rng(0)
    demo = {
        "x": rng.standard_normal((NB, L, D)).astype(np.float32),
        "w_qkv": rng.standard_normal((D, 3 * D)).astype(np.float32) * 0.03,
        "b_qkv": rng.standard_normal(3 * D).astype(np.float32) * 0.03,
        "w_o": rng.standard_normal((D, D)).astype(np.float32) * 0.03,
        "b_o": rng.standard_normal(D).astype(np.float32) * 0.03,
        "g1": np.ones(D, np.float32), "be1": np.zeros(D, np.float32),
        "w1": rng.standard_normal((D, FF)).astype(np.float32) * 0.03,
        "b1": rng.standard_normal(FF).astype(np.float32) * 0.03,
        "w2": rng.standard_normal((FF, D)).astype(np.float32) * 0.015,
        "b2": rng.standard_normal(D).astype(np.float32) * 0.015,
        "g2": np.ones(D, np.float32), "be2": np.zeros(D, np.float32),
    }
    out = kernel(**demo)
    print("kernel output:", out.shape, out.dtype, np.abs(out).mean())



# revision 3
# speedup vs baseline: 1.5720x; 1.5720x over previous
"""Trainium2 Bass kernel for a transformer encoder layer (nn_Encoder).

x:[2,2048,1024] f32. 8 NeuronCores, data-parallel: core c handles batch
n=c//4, query rows qi=c%4 (512 tokens). K/V are recomputed per core for
the full batch (x4 redundancy) to avoid collectives (~300us for the
8.4MB all-reduce this would replace).

All matmuls run in fp8 e4m3 with DoubleRow perf mode where the
contraction is >=256 (2x PE column rate AND 256-deep contraction per
instruction => 4x fewer PE cycles than f32r for the projections/FFN).
Weights are host-scaled x16 so their U(-1/32,1/32) range lands in the
fp8 normal range; the scale is unwound via the softmax ones-row (=16),
the exp scale (0.125/256 for q,k both x16) and a /256 on the fc2 PSUM.
Residuals/LayerNorm stay f32. exp (ACT) is the critical engine.
"""
import os
import sys
from contextlib import ExitStack

for _p in ("/opt/trn_rl_repo", "/root/.axon_site/_ro/trn_rl_repo"):
    if os.path.isdir(_p) and _p not in sys.path:
        sys.path.insert(0, _p)

import numpy as np
import concourse.bass as bass
import concourse.mybir as mybir
import concourse.tile as tile
from concourse import bacc
from concourse.bass_utils import run_bass_kernel_spmd
from concourse.masks import make_identity

F32 = mybir.dt.float32
F32R = mybir.dt.float32r
F8 = mybir.dt.float8e4
AF = mybir.ActivationFunctionType
ALU = mybir.AluOpType
DR = mybir.MatmulPerfMode.DoubleRow

D = 1024
H = 16
HD = 64
FF = 4096
L = 2048
NB = 2
P = 128
QT = 512          # query tokens per core
DC = D // P       # 8 chunks of the model dim
DC2 = DC // 2     # 4 DoubleRow chunk-pairs
KT = L // P       # 16 key tiles
FC = FF // P      # 32 ff chunks
FC2 = FC // 2     # 16 ff chunk-pairs
TT = QT // P      # 4 own-token tiles
NPAIR = H // 2    # 8 head pairs
NP2 = NPAIR // 2  # 4 pair-pairs
EPS = 1e-5
WS = 16.0         # host weight scale (fp8 range)

_CACHED_NC = {}


def _build_nc(affine=True):
    nc = bacc.Bacc("TRN2", target_bir_lowering=False)

    def dparam(name, shape, dt=F8):
        return nc.dram_tensor(name, shape, dt, kind="ExternalInput")

    xT8 = dparam("xT8", [D, L])            # x[n].T, fp8
    xTq8 = dparam("xTq8", [D, QT])         # own-token columns of xT, fp8
    xq = dparam("xq", [QT, D], F32)        # own tokens, natural (residual)
    # weights: partition-major fp8, DoubleRow k-tile-pair layouts
    wq8 = dparam("wq8", [P, NPAIR, 2 * DC2 * P])     # [p][pr][d2 i m]
    wk8 = dparam("wk8", [P, NPAIR, 2 * DC2 * P])
    wv8 = dparam("wv8", [P, 2 * DC2 * 2 * 512])      # [p][vh d2 i n]
    wo8 = dparam("wo8", [P, NP2 * 2 * D])            # [p][j i o]
    w18 = dparam("w18", [P, FC // 4, 4 * DC2 * 2 * P])   # [p][f4][f d2 i m]
    w28 = dparam("w28", [P, FC2 // 2, 2 * 2 * D])        # [p][s][g i o]
    bq = dparam("bq", [P, NPAIR], F32)     # x16 biases
    bk = dparam("bk", [P, NPAIR], F32)
    b1 = dparam("b1", [P, FC], F32)
    bvb = dparam("bvb", [P, D], F32)
    b2b = dparam("b2b", [P, D], F32)       # natural scale
    g1b = dparam("g1b", [P, D], F32)
    be1b = dparam("be1b", [P, D], F32)
    g2b = dparam("g2b", [P, D], F32)
    be2b = dparam("be2b", [P, D], F32)
    vones = dparam("vones", [P, KT])       # value 16 (denominator row)

    y = nc.dram_tensor("y", [QT, D], F32, kind="ExternalOutput")

    with tile.TileContext(nc) as tc:
        with tc.tile_pool(name="pers", bufs=1) as pers:
            ident = pers.tile([P, P], F32R, tag="ident")
            make_identity(nc, ident[:])
            bq_t = pers.tile([P, NPAIR], F32, tag="bq")
            bk_t = pers.tile([P, NPAIR], F32, tag="bk")
            b1_t = pers.tile([P, FC], F32, tag="b1")

            xT_t = pers.tile([P, DC, L], F8, tag="xT")
            v_aug = pers.tile([P, KT, H * (HD + 1)], F8, tag="vaug")
            ones_t = pers.tile([P, KT], F8, tag="ones")
            xTq_t = pers.tile([P, DC, QT], F8, tag="xTq")
            nc.sync.dma_start(xTq_t[:], xTq8.rearrange("(c p) t -> p c t", p=P))
            nc.scalar.dma_start(bq_t[:], bq[:])
            nc.scalar.dma_start(bk_t[:], bk[:])
            nc.scalar.dma_start(b1_t[:], b1[:])

            # ================= Q projection (own tokens) ==============
            qT_t = pers.tile([P, NPAIR, QT], F8, tag="qT")
            with tc.tile_pool(name="qp", bufs=2) as qp, \
                 tc.tile_pool(name="qps", bufs=4, space="PSUM") as qps:
                for pr2 in range(NPAIR // 2):
                    wq_t = qp.tile([P, 2, DC2, 2 * P], F8, tag="w")
                    nc.scalar.dma_start(
                        wq_t[:], wq8[:, 2 * pr2:2 * pr2 + 2, :].rearrange(
                            "p r (c m) -> p r c m", c=DC2))
                    for r in range(2):
                        pr = 2 * pr2 + r
                        ps = qps.tile([P, 512], F32, tag="mm")
                        for d2 in range(DC2):
                            nc.tensor.matmul(
                                ps[:],
                                wq_t[:, r, d2, :].rearrange("p (i m) -> p i m", i=2),
                                xTq_t[:, 2 * d2:2 * d2 + 2, :],
                                start=(d2 == 0), stop=(d2 == DC2 - 1),
                                perf_mode=DR)
                        nc.vector.tensor_scalar(qT_t[:, pr, :], ps[:],
                                                bq_t[:, pr:pr + 1], None, ALU.add)

            for dc in range(DC):
                nc.sync.dma_start(
                    xT_t[:, dc, :],
                    xT8.rearrange("(c p) t -> p c t", p=P)[:, dc, :])
            nc.scalar.dma_start(ones_t[:], vones[:])
            nc.vector.tensor_copy(
                v_aug[:].rearrange("p t (h c) -> p t h c", c=HD + 1)[:, :, :, HD],
                ones_t[:, :, None].to_broadcast([P, KT, H]))

            # ================= V projection ===========================
            with tc.tile_pool(name="vp", bufs=1) as vp, \
                 tc.tile_pool(name="vps", bufs=1, space="PSUM") as vps:
                bvb_t = vp.tile([P, D], F32, tag="bvb")
                nc.scalar.dma_start(bvb_t[:], bvb[:])
                wv_t = vp.tile([P, 2, DC2, 2 * 512], F8, tag="wv")
                nc.scalar.dma_start(
                    wv_t[:], wv8.rearrange("p (v c m) -> p v c m", v=2, c=DC2))
                for vh in range(2):
                    for pas in range(4):
                        ps4 = [vps.tile([P, 512], F32, tag=f"vmm{i}",
                                        name=f"vps_{vh}_{pas}_{i}") for i in range(4)]
                        for d2 in range(DC2):
                            for i in range(4):
                                tt = pas * 4 + i
                                nc.tensor.matmul(
                                    ps4[i],
                                    xT_t[:, 2 * d2:2 * d2 + 2, tt * P:(tt + 1) * P],
                                    wv_t[:, vh, d2, :].rearrange(
                                        "p (i n) -> p i n", i=2),
                                    start=(d2 == 0), stop=(d2 == DC2 - 1),
                                    perf_mode=DR)
                        for i in range(4):
                            tt = pas * 4 + i
                            dst = v_aug[:, tt, :].rearrange(
                                "p (h c) -> p h c", c=HD + 1)[:, vh * 8:(vh + 1) * 8, 0:HD]
                            nc.vector.tensor_tensor(
                                dst, ps4[i].rearrange("p (h c) -> p h c", c=HD),
                                bvb_t[:, vh * 512:(vh + 1) * 512].rearrange(
                                    "p (h c) -> p h c", c=HD),
                                ALU.add)

            # ========== K projection fused with attention, per pair ====
            # PSUM: K accum 2 + S^T 2x2 + PV accum 2 = 8 banks exactly.
            outSB = pers.tile([P, NPAIR, QT], F8, tag="outSB")
            with tc.tile_pool(name="kp", bufs=2) as kp, \
                 tc.tile_pool(name="kq", bufs=2) as kq, \
                 tc.tile_pool(name="atp", bufs=3) as atp, \
                 tc.tile_pool(name="atd", bufs=2) as atd, \
                 tc.tile_pool(name="kps", bufs=1, space="PSUM") as kps, \
                 tc.tile_pool(name="stp", bufs=2, space="PSUM") as stpool, \
                 tc.tile_pool(name="pvp", bufs=2, space="PSUM") as pvpool:
                for pr in range(NPAIR):
                    wk_t = kp.tile([P, DC2, 2 * P], F8, tag="w")
                    eng = nc.scalar if pr % 2 else nc.sync
                    eng.dma_start(wk_t[:], wk8[:, pr, :].rearrange(
                        "p (c m) -> p c m", c=DC2))
                    kT_sb = kq.tile([P, L], F8, tag="kts", name=f"kts_{pr}")
                    for half in range(2):
                        pst = [kps.tile([P, 512], F32, tag=f"kmm{i}",
                                        name=f"kps_{pr}_{half}_{i}") for i in range(2)]
                        for d2 in range(DC2):
                            for i in range(2):
                                t4 = half * 2 + i
                                nc.tensor.matmul(
                                    pst[i],
                                    wk_t[:, d2, :].rearrange("p (i m) -> p i m", i=2),
                                    xT_t[:, 2 * d2:2 * d2 + 2, t4 * 512:(t4 + 1) * 512],
                                    start=(d2 == 0), stop=(d2 == DC2 - 1),
                                    perf_mode=DR)
                        for i in range(2):
                            t4 = half * 2 + i
                            nc.vector.tensor_scalar(
                                kT_sb[:, t4 * 512:(t4 + 1) * 512], pst[i],
                                bk_t[:, pr:pr + 1], None, ALU.add)
                    # ---- attention for this pair ----
                    pvs = [pvpool.tile([P, QT], F32, tag="pv", name=f"pv_{pr}_{h2}")
                           for h2 in range(2)]
                    for gi in range(KT // 2):
                        a = 2 * gi
                        for h2 in range(2):
                            h_idx = 2 * pr + h2
                            rows = slice(h2 * HD, h2 * HD + HD)
                            stp = stpool.tile([P, 1024], F32, tag="st",
                                              name=f"st_{pr}_{a}_{h2}")
                            for j in range(2):
                                kt = a + j
                                nc.tensor.matmul(
                                    stp[:, j * 512:(j + 1) * 512],
                                    kT_sb[rows, kt * P:(kt + 1) * P],
                                    qT_t[rows, pr, :], start=True, stop=True)
                            ptt = atp.tile([P, 2, QT], F8, tag="pt",
                                           name=f"pt_{pr}_{a}_{h2}")
                            nc.scalar.activation(
                                ptt[:],
                                stp[:].rearrange("p (c n) -> p c n", n=512),
                                AF.Exp, scale=0.125 / (WS * WS))
                            nc.tensor.matmul(
                                pvs[h2][:HD + 1, :],
                                v_aug[:, a:a + 2, h_idx * (HD + 1):(h_idx + 1) * (HD + 1)],
                                ptt[:],
                                start=(gi == 0), stop=(gi == KT // 2 - 1),
                                perf_mode=DR)
                    for h2 in range(2):
                        rows = slice(h2 * HD, h2 * HD + HD)
                        den = atd.tile([1, QT], F32, tag="den", name=f"den_{pr}_{h2}")
                        nc.vector.reciprocal(den[:], pvs[h2][HD:HD + 1, :])
                        denb = atd.tile([HD, QT], F32, tag="denb",
                                        name=f"denb_{pr}_{h2}")
                        nc.gpsimd.partition_broadcast(denb[:], den[:])
                        nc.vector.tensor_tensor(outSB[rows, pr, :], pvs[h2][:HD, :],
                                                denb[:], ALU.mult)

            # ================= Output proj + residual + LN1 ===========
            h_t = pers.tile([P, TT, D], F32R, tag="h")
            hT_t = pers.tile([P, DC, QT], F8, tag="hT")
            with tc.tile_pool(name="op", bufs=1) as op, \
                 tc.tile_pool(name="lnw", bufs=1) as lnw, \
                 tc.tile_pool(name="lnp3", bufs=2) as lnp3, \
                 tc.tile_pool(name="ops", bufs=1, space="PSUM") as ops, \
                 tc.tile_pool(name="tps", bufs=2, space="PSUM") as tps:
                if affine:
                    g1b_t = lnw.tile([P, D], F32, tag="g1b")
                    be1b_t = lnw.tile([P, D], F32, tag="be1b")
                    nc.scalar.dma_start(g1b_t[:], g1b[:])
                    nc.scalar.dma_start(be1b_t[:], be1b[:])
                else:
                    g1b_t = be1b_t = None
                wo_t = op.tile([P, NP2, 2 * D], F8, tag="wof")
                nc.scalar.dma_start(wo_t[:], wo8.rearrange("p (j m) -> p j m", j=NP2))
                xq_s = op.tile([P, TT, D], F32, tag="xqs")
                nc.sync.dma_start(xq_s[:], xq.rearrange("(t p) d -> p t d", p=P))
                for half in range(2):
                    tts = (2 * half, 2 * half + 1)
                    pso = [ops.tile([P, 512], F32, tag=f"ao{i}",
                                    name=f"ao_{half}_{i}") for i in range(4)]
                    for j in range(NP2):
                        for i, tt in enumerate(tts):
                            for oc in range(2):
                                nc.tensor.matmul(
                                    pso[i * 2 + oc],
                                    outSB[:, 2 * j:2 * j + 2, tt * P:(tt + 1) * P],
                                    wo_t[:, j, :].rearrange(
                                        "p (i o) -> p i o", i=2)[:, :, oc * 512:(oc + 1) * 512],
                                    start=(j == 0), stop=(j == NP2 - 1),
                                    perf_mode=DR)
                    for i, tt in enumerate(tts):
                        for oc in range(2):
                            nc.vector.tensor_tensor(
                                h_t[:, tt, oc * 512:(oc + 1) * 512],
                                pso[i * 2 + oc],
                                xq_s[:, tt, oc * 512:(oc + 1) * 512], ALU.add)
                        _layernorm(nc, lnp3, h_t[:, tt, :], h_t[:, tt, :],
                                   g1b_t, be1b_t, affine)
                        for dcg in range(2):
                            pst = tps.tile([P, 4, P], F32R, tag="tp",
                                           name=f"tp_{tt}_{dcg}")
                            for k in range(4):
                                dc = 4 * dcg + k
                                nc.tensor.transpose(
                                    pst[:, k, :], h_t[:, tt, dc * P:(dc + 1) * P],
                                    ident[:])
                            nc.vector.tensor_copy(
                                hT_t[:, 4 * dcg:4 * dcg + 4, tt * P:(tt + 1) * P],
                                pst[:])

            # ================= FFN + LN2 ==============================
            ff1 = pers.tile([P, FC, QT], F8, tag="ff1")
            with tc.tile_pool(name="fp", bufs=2) as fp, \
                 tc.tile_pool(name="fw", bufs=1) as fw, \
                 tc.tile_pool(name="ft", bufs=2) as ft, \
                 tc.tile_pool(name="lnp4", bufs=2) as lnp4:
                with tc.tile_pool(name="f1s", bufs=4, space="PSUM") as f1s:
                    for fc4 in range(FC // 4):
                        w1_t = fp.tile([P, 4, DC2, 2 * P], F8, tag="wstream")
                        eng = nc.scalar if fc4 % 2 else nc.sync
                        eng.dma_start(w1_t[:], w18[:, fc4, :].rearrange(
                            "p (f c m) -> p f c m", f=4, c=DC2))
                        for f in range(4):
                            fc = 4 * fc4 + f
                            ps = f1s.tile([P, 512], F32, tag="mm")
                            for d2 in range(DC2):
                                nc.tensor.matmul(
                                    ps[:],
                                    w1_t[:, f, d2, :].rearrange("p (i m) -> p i m", i=2),
                                    hT_t[:, 2 * d2:2 * d2 + 2, :],
                                    start=(d2 == 0), stop=(d2 == DC2 - 1),
                                    perf_mode=DR)
                            # fused bias + relu (x16 scale kept; /256 in fc2)
                            nc.vector.tensor_scalar(ff1[:, fc, :], ps[:],
                                                    b1_t[:, fc:fc + 1], 0.0,
                                                    ALU.add, ALU.max)

                b2b_t = fw.tile([P, D], F32, tag="b2b")
                nc.scalar.dma_start(b2b_t[:], b2b[:])
                # fold the fc2 bias into the residual while ff1 runs
                for tt in range(TT):
                    nc.vector.tensor_tensor(h_t[:, tt, :], h_t[:, tt, :],
                                            b2b_t[:], ALU.add)
                if affine:
                    g2b_t = fw.tile([P, D], F32, tag="g2b")
                    be2b_t = fw.tile([P, D], F32, tag="be2b")
                    nc.scalar.dma_start(g2b_t[:], g2b[:])
                    nc.scalar.dma_start(be2b_t[:], be2b[:])
                else:
                    g2b_t = be2b_t = None
                with tc.tile_pool(name="f2s", bufs=1, space="PSUM") as f2s:
                    pss = [f2s.tile([P, 512], F32, tag=f"ff2_{i}", name=f"ff2_{i}")
                           for i in range(8)]
                    for s in range(FC2 // 2):
                        w2_t = fp.tile([P, 2, 2, D], F8, tag="wstream")
                        eng = nc.scalar if s % 2 else nc.sync
                        eng.dma_start(w2_t[:], w28[:, s, :].rearrange(
                            "p (g i o) -> p g i o", g=2, i=2))
                        for g in range(2):
                            f2 = 2 * s + g
                            for tt in range(TT):
                                for oc in range(2):
                                    nc.tensor.matmul(
                                        pss[tt * 2 + oc],
                                        ff1[:, 2 * f2:2 * f2 + 2, tt * P:(tt + 1) * P],
                                        w2_t[:, g, :, oc * 512:(oc + 1) * 512],
                                        start=(f2 == 0), stop=(f2 == FC2 - 1),
                                        perf_mode=DR)
                    for tt in range(TT):
                        t2 = ft.tile([P, D], F32, tag="t2")
                        for oc in range(2):
                            # (psum / 256) + h  (undo w1*16 * w2*16)
                            nc.vector.scalar_tensor_tensor(
                                t2[:, oc * 512:(oc + 1) * 512],
                                pss[tt * 2 + oc], 1.0 / (WS * WS),
                                h_t[:, tt, oc * 512:(oc + 1) * 512],
                                ALU.mult, ALU.add)
                        _layernorm(nc, lnp4, t2[:], t2[:], g2b_t, be2b_t, affine)
                        nc.sync.dma_start(
                            y.rearrange("(t p) d -> p t d", p=P)[:, tt, :], t2[:])

    nc.compile()
    return nc


def _layernorm(nc, pool, dst, src, g_t, be_t, affine):
    """dst = (src - mean)/sqrt(var + eps) [* g + be], row-wise over 1024.

    bn_stats/bn_aggr produce mean+var in one DVE pass. rsqrt is computed
    as exp(-0.5*ln(v)) on ACT (both funcs live in one activation table,
    so no table thrash with the attention exps) and refined with one
    Newton step on DVE.
    """
    stats = pool.tile([P, 2, 6], F32, tag="ln_st")
    nc.vector.bn_stats(stats[:, 0, :], src[:, 0:D // 2])
    nc.vector.bn_stats(stats[:, 1, :], src[:, D // 2:D])
    mv = pool.tile([P, 2], F32, tag="ln_mv")
    nc.vector.bn_aggr(mv[:], stats[:])
    vv = pool.tile([P, 1], F32, tag="ln_v")
    nc.vector.tensor_scalar(vv[:], mv[:, 1:2], EPS, None, ALU.add)
    lnv = pool.tile([P, 1], F32, tag="ln_ln")
    nc.scalar.activation(lnv[:], vv[:], AF.Ln)
    r = pool.tile([P, 1], F32, tag="ln_r")
    nc.scalar.activation(r[:], lnv[:], AF.Exp, scale=-0.5)
    # one Newton step: r <- r * (1.5 - 0.5 * vv * r^2)
    t = pool.tile([P, 1], F32, tag="ln_t")
    nc.vector.tensor_tensor(t[:], r[:], r[:], ALU.mult)
    nc.vector.tensor_tensor(t[:], t[:], vv[:], ALU.mult)
    nc.vector.tensor_scalar(t[:], t[:], -0.5, 1.5, ALU.mult, ALU.add)
    nc.vector.tensor_tensor(r[:], r[:], t[:], ALU.mult)
    nc.vector.tensor_scalar(dst, src, mv[:, 0:1], r[:], ALU.subtract, ALU.mult)
    if affine:
        nc.vector.tensor_tensor(dst, dst, g_t[:], ALU.mult)
        nc.vector.tensor_tensor(dst, dst, be_t[:], ALU.add)


def make_in_maps(x, w_qkv, b_qkv, w_o, b_o, g1, be1, w1, b1, w2, b2, g2, be2):
    f = np.float32
    f8 = mybir.dt.np(F8)
    x = np.asarray(x, f)
    w_qkv = np.asarray(w_qkv, f)
    b_qkv = np.asarray(b_qkv, f)
    bc = lambda v: np.ascontiguousarray(
        np.broadcast_to(np.asarray(v, f).reshape(1, D), (P, D)))

    # [d, m] -> [p, pr, (d2 i m)] partition-major DoubleRow layout
    def qk_layout(w):
        return np.ascontiguousarray(
            (w * WS).reshape(DC2, 2, P, NPAIR, P).transpose(2, 3, 0, 1, 4)
            .reshape(P, NPAIR, 2 * DC2 * P)).astype(f8)

    wv_h = np.ascontiguousarray(
        (w_qkv[:, 2 * D:] * WS).reshape(DC2, 2, P, 2, 512)
        .transpose(2, 3, 0, 1, 4).reshape(P, 2 * DC2 * 2 * 512)).astype(f8)
    wo_h = np.ascontiguousarray(
        np.asarray(w_o, f).reshape(NP2, 2, P, D).transpose(2, 0, 1, 3)
        .reshape(P, NP2 * 2 * D)).astype(f8)
    w1_h = np.ascontiguousarray(
        (np.asarray(w1, f) * WS).reshape(DC2, 2, P, FC // 4, 4, P)
        .transpose(2, 3, 4, 0, 1, 5).reshape(P, FC // 4, 4 * DC2 * 2 * P)).astype(f8)
    w2_h = np.ascontiguousarray(
        (np.asarray(w2, f) * WS).reshape(FC2 // 2, 2, 2, P, D)
        .transpose(3, 0, 1, 2, 4).reshape(P, FC2 // 2, 2 * 2 * D)).astype(f8)

    shared = {
        "wq8": qk_layout(w_qkv[:, :D]),
        "wk8": qk_layout(w_qkv[:, D:2 * D]),
        "wv8": wv_h, "wo8": wo_h, "w18": w1_h, "w28": w2_h,
        "bq": np.ascontiguousarray((b_qkv[:D] * WS).reshape(NPAIR, P).T),
        "bk": np.ascontiguousarray((b_qkv[D:2 * D] * WS).reshape(NPAIR, P).T),
        "b1": np.ascontiguousarray((np.asarray(b1, f) * WS).reshape(FC, P).T),
        "bvb": bc(np.asarray(b_qkv[2 * D:], f) * WS), "b2b": bc(b2),
        "g1b": bc(g1), "be1b": bc(be1), "g2b": bc(g2), "be2b": bc(be2),
        "vones": np.full((P, KT), WS, f).astype(f8),
    }
    in_maps = []
    for c in range(8):
        n, qi = divmod(c, 4)
        xT8n = np.ascontiguousarray(x[n].T).astype(f8)
        m = dict(shared)
        m["xT8"] = xT8n
        m["xTq8"] = np.ascontiguousarray(xT8n[:, qi * QT:(qi + 1) * QT])
        m["xq"] = np.ascontiguousarray(x[n, qi * QT:(qi + 1) * QT, :]
                                       + np.asarray(b_o, f).reshape(1, D))
        in_maps.append(m)
    return in_maps


def get_nc(affine=True):
    if affine not in _CACHED_NC:
        _CACHED_NC[affine] = _build_nc(affine)
    return _CACHED_NC[affine]


def kernel(**inputs):
    in_maps = make_in_maps(**inputs)
    affine = not (np.all(np.asarray(inputs["g1"]) == 1)
                  and np.all(np.asarray(inputs["be1"]) == 0)
                  and np.all(np.asarray(inputs["g2"]) == 1)
                  and np.all(np.asarray(inputs["be2"]) == 0))
    nc = get_nc(affine)
    # The axon-proxied NRT occasionally reports a transient
    # NRT_EXEC_UNIT_UNRECOVERABLE on a cold first dispatch; a plain retry
    # has always succeeded with bit-identical results, so recover inline.
    last_err = None
    for _ in range(3):
        try:
            res = run_bass_kernel_spmd(nc, in_maps, list(range(8))).results
            break
        except Exception as e:  # noqa: BLE001
            last_err = e
    else:
        raise last_err
    yout = np.empty((NB, L, D), np.float32)
    for c in range(8):
        n, qi = divmod(c, 4)
        yout[n, qi * QT:(qi + 1) * QT] = res[c]["y"]
    return yout


if __name__ == "__main__":
    rng = np.random.default_rng(0)
    demo = {
        "x": rng.standard_normal((NB, L, D)).astype(np.float32),
        "w_qkv": rng.standard_normal((D, 3 * D)).astype(np.float32) * 0.03,
        "b_qkv": rng.standard_normal(3 * D).astype(np.float32) * 0.03,
        "w_o": rng.standard_normal((D, D)).astype(np.float32) * 0.03,
        "b_o": rng.standard_normal(D).astype(np.float32) * 0.03,
        "g1": np.ones(D, np.float32), "be1": np.zeros(D, np.float32),
        "w1": rng.standard_normal((D, FF)).astype(np.float32) * 0.03,
        "b1": rng.standard_normal(FF).astype(np.float32) * 0.03,
        "w2": rng.standard_normal((FF, D)).astype(np.float32) * 0.015,
        "b2": rng.standard_normal(D).astype(np.float32) * 0.015,
        "g2": np.ones(D, np.float32), "be2": np.zeros(D, np.float32),
    }
    out = kernel(**demo)
    print("kernel output:", out.shape, out.dtype, np.abs(out).mean())


# revision 7
# speedup vs baseline: 1.5947x; 1.0144x over previous
"""Trainium2 Bass kernel for a transformer encoder layer (nn_Encoder).

x:[2,2048,1024] f32. 8 NeuronCores, data-parallel: core c handles batch
n=c//4, query rows qi=c%4 (512 tokens). K/V are recomputed per core for
the full batch (x4 redundancy) to avoid collectives (~300us for the
8.4MB all-reduce this would replace).

All matmuls run in fp8 e4m3 with DoubleRow perf mode where the
contraction is >=256 (2x PE column rate AND 256-deep contraction per
instruction => 4x fewer PE cycles than f32r for the projections/FFN).
Weights are host-scaled x16 so their U(-1/32,1/32) range lands in the
fp8 normal range; the scale is unwound via the softmax ones-row (=16),
the exp scale (0.125/256 for q,k both x16) and a /256 on the fc2 PSUM.
Residuals/LayerNorm stay f32. exp (ACT) is the critical engine.
"""
import os
import sys
from contextlib import ExitStack

for _p in ("/opt/trn_rl_repo", "/root/.axon_site/_ro/trn_rl_repo"):
    if os.path.isdir(_p) and _p not in sys.path:
        sys.path.insert(0, _p)

import numpy as np
import concourse.bass as bass
import concourse.mybir as mybir
import concourse.tile as tile
from concourse import bacc
from concourse.bass_utils import run_bass_kernel_spmd
from concourse.masks import make_identity

F32 = mybir.dt.float32
F32R = mybir.dt.float32r
F8 = mybir.dt.float8e4
AF = mybir.ActivationFunctionType
ALU = mybir.AluOpType
DR = mybir.MatmulPerfMode.DoubleRow

D = 1024
H = 16
HD = 64
FF = 4096
L = 2048
NB = 2
P = 128
QT = 512          # query tokens per core
DC = D // P       # 8 chunks of the model dim
DC2 = DC // 2     # 4 DoubleRow chunk-pairs
KT = L // P       # 16 key tiles
FC = FF // P      # 32 ff chunks
FC2 = FC // 2     # 16 ff chunk-pairs
TT = QT // P      # 4 own-token tiles
NPAIR = H // 2    # 8 head pairs
NP2 = NPAIR // 2  # 4 pair-pairs
EPS = 1e-5
WS = 16.0         # host weight scale (fp8 range)

_CACHED_NC = {}


def _build_nc(affine=True):
    nc = bacc.Bacc("TRN2", target_bir_lowering=False)

    def dparam(name, shape, dt=F8):
        return nc.dram_tensor(name, shape, dt, kind="ExternalInput")

    xT8 = dparam("xT8", [D, L])            # x[n].T, fp8
    xTq8 = dparam("xTq8", [D, QT])         # own-token columns of xT, fp8
    xq = dparam("xq", [QT, D], F32)        # own tokens, natural (residual)
    # weights: partition-major fp8, DoubleRow k-tile-pair layouts
    wq8 = dparam("wq8", [P, NPAIR, 2 * DC2 * P])     # [p][pr][d2 i m]
    wk8 = dparam("wk8", [P, NPAIR, 2 * DC2 * P])
    wv8 = dparam("wv8", [P, 2 * DC2 * 2 * 512])      # [p][vh d2 i n]
    wo8 = dparam("wo8", [P, NP2 * 2 * D])            # [p][j i o]
    w18 = dparam("w18", [P, FC // 4, 4 * DC2 * 2 * P])   # [p][f4][f d2 i m]
    w28 = dparam("w28", [P, FC2 // 2, 2 * 2 * D])        # [p][s][g i o]
    bq = dparam("bq", [P, NPAIR], F32)     # x16 biases
    bk = dparam("bk", [P, NPAIR], F32)
    b1 = dparam("b1", [P, FC], F32)
    bvb = dparam("bvb", [P, D], F32)
    b2b = dparam("b2b", [P, D], F32)       # natural scale
    g1b = dparam("g1b", [P, D], F32)
    be1b = dparam("be1b", [P, D], F32)
    g2b = dparam("g2b", [P, D], F32)
    be2b = dparam("be2b", [P, D], F32)
    vones = dparam("vones", [P, KT])       # value 16 (denominator row)

    y = nc.dram_tensor("y", [QT, D], F32, kind="ExternalOutput")

    with tile.TileContext(nc) as tc:
        # Pre-load the one activation table that serves every ACT func we
        # use (Exp for softmax, Ln+Exp for the LN rsqrt): without this the
        # first-fit chooser thrashes exp<->ln tables at 1283ns per load.
        from concourse.hw_specs import get_activation_tables
        _tabs = get_activation_tables(nc.m.arch)
        _idx = next(i for i, (_, s) in enumerate(_tabs.items())
                    if AF.Exp in s and AF.Ln in s)
        nc.scalar.add_instruction(mybir.InstLoadActFuncSet(
            name=nc.scalar.bass.get_next_instruction_name(),
            act_func_set_id=_idx, ins=[], outs=[]))
        with tc.tile_pool(name="pers", bufs=1) as pers:
            ident = pers.tile([P, P], F32, tag="ident")
            make_identity(nc, ident[:])
            bq_t = pers.tile([P, NPAIR], F32, tag="bq")
            bk_t = pers.tile([P, NPAIR], F32, tag="bk")
            b1_t = pers.tile([P, FC], F32, tag="b1")

            xT_t = pers.tile([P, DC, L], F8, tag="xT")
            v_aug = pers.tile([P, KT, H * (HD + 1)], F8, tag="vaug")
            ones_t = pers.tile([P, KT], F8, tag="ones")
            xTq_t = pers.tile([P, DC, QT], F8, tag="xTq")
            nc.sync.dma_start(xTq_t[:], xTq8.rearrange("(c p) t -> p c t", p=P))
            nc.scalar.dma_start(bq_t[:], bq[:])
            nc.scalar.dma_start(bk_t[:], bk[:])
            nc.scalar.dma_start(b1_t[:], b1[:])

            # ================= Q projection (own tokens) ==============
            qT_t = pers.tile([P, NPAIR, QT], F8, tag="qT")
            with tc.tile_pool(name="qp", bufs=2) as qp, \
                 tc.tile_pool(name="qps", bufs=4, space="PSUM") as qps:
                for pr2 in range(NPAIR // 2):
                    wq_t = qp.tile([P, 2, DC2, 2 * P], F8, tag="w")
                    nc.scalar.dma_start(
                        wq_t[:], wq8[:, 2 * pr2:2 * pr2 + 2, :].rearrange(
                            "p r (c m) -> p r c m", c=DC2))
                    for r in range(2):
                        pr = 2 * pr2 + r
                        ps = qps.tile([P, 512], F32, tag="mm")
                        for d2 in range(DC2):
                            nc.tensor.matmul(
                                ps[:],
                                wq_t[:, r, d2, :].rearrange("p (i m) -> p i m", i=2),
                                xTq_t[:, 2 * d2:2 * d2 + 2, :],
                                start=(d2 == 0), stop=(d2 == DC2 - 1),
                                perf_mode=DR)
                        nc.vector.tensor_scalar(qT_t[:, pr, :], ps[:],
                                                bq_t[:, pr:pr + 1], None, ALU.add)

            for dc in range(DC):
                nc.sync.dma_start(
                    xT_t[:, dc, :],
                    xT8.rearrange("(c p) t -> p c t", p=P)[:, dc, :])
            nc.scalar.dma_start(ones_t[:], vones[:])
            nc.vector.tensor_copy(
                v_aug[:].rearrange("p t (h c) -> p t h c", c=HD + 1)[:, :, :, HD],
                ones_t[:, :, None].to_broadcast([P, KT, H]))

            # ================= V projection ===========================
            with tc.tile_pool(name="vp", bufs=1) as vp, \
                 tc.tile_pool(name="vps", bufs=1, space="PSUM") as vps:
                bvb_t = vp.tile([P, D], F32, tag="bvb")
                nc.scalar.dma_start(bvb_t[:], bvb[:])
                wv_t = vp.tile([P, 2, DC2, 2 * 512], F8, tag="wv")
                nc.scalar.dma_start(
                    wv_t[:], wv8.rearrange("p (v c m) -> p v c m", v=2, c=DC2))
                for vh in range(2):
                    for pas in range(4):
                        ps4 = [vps.tile([P, 512], F32, tag=f"vmm{i}",
                                        name=f"vps_{vh}_{pas}_{i}") for i in range(4)]
                        for d2 in range(DC2):
                            for i in range(4):
                                tt = pas * 4 + i
                                nc.tensor.matmul(
                                    ps4[i],
                                    xT_t[:, 2 * d2:2 * d2 + 2, tt * P:(tt + 1) * P],
                                    wv_t[:, vh, d2, :].rearrange(
                                        "p (i n) -> p i n", i=2),
                                    start=(d2 == 0), stop=(d2 == DC2 - 1),
                                    perf_mode=DR)
                        for i in range(4):
                            tt = pas * 4 + i
                            dst = v_aug[:, tt, :].rearrange(
                                "p (h c) -> p h c", c=HD + 1)[:, vh * 8:(vh + 1) * 8, 0:HD]
                            nc.vector.tensor_tensor(
                                dst, ps4[i].rearrange("p (h c) -> p h c", c=HD),
                                bvb_t[:, vh * 512:(vh + 1) * 512].rearrange(
                                    "p (h c) -> p h c", c=HD),
                                ALU.add)

            # ========== K projection fused with attention, per pair ====
            # PSUM: K accum 2 + S^T 2x2 + PV accum 2 = 8 banks exactly.
            outSB = pers.tile([P, NPAIR, QT], F8, tag="outSB")
            with tc.tile_pool(name="kp", bufs=2) as kp, \
                 tc.tile_pool(name="kq", bufs=2) as kq, \
                 tc.tile_pool(name="atp", bufs=3) as atp, \
                 tc.tile_pool(name="atd", bufs=2) as atd, \
                 tc.tile_pool(name="kps", bufs=1, space="PSUM") as kps, \
                 tc.tile_pool(name="stp", bufs=2, space="PSUM") as stpool, \
                 tc.tile_pool(name="pvp", bufs=2, space="PSUM") as pvpool:
                for pr in range(NPAIR):
                    wk_t = kp.tile([P, DC2, 2 * P], F8, tag="w")
                    eng = nc.scalar if pr % 2 else nc.sync
                    eng.dma_start(wk_t[:], wk8[:, pr, :].rearrange(
                        "p (c m) -> p c m", c=DC2))
                    kT_sb = kq.tile([P, L], F8, tag="kts", name=f"kts_{pr}")
                    for half in range(2):
                        pst = [kps.tile([P, 512], F32, tag=f"kmm{i}",
                                        name=f"kps_{pr}_{half}_{i}") for i in range(2)]
                        for d2 in range(DC2):
                            for i in range(2):
                                t4 = half * 2 + i
                                nc.tensor.matmul(
                                    pst[i],
                                    wk_t[:, d2, :].rearrange("p (i m) -> p i m", i=2),
                                    xT_t[:, 2 * d2:2 * d2 + 2, t4 * 512:(t4 + 1) * 512],
                                    start=(d2 == 0), stop=(d2 == DC2 - 1),
                                    perf_mode=DR)
                        for i in range(2):
                            t4 = half * 2 + i
                            nc.vector.tensor_scalar(
                                kT_sb[:, t4 * 512:(t4 + 1) * 512], pst[i],
                                bk_t[:, pr:pr + 1], None, ALU.add)
                    # ---- attention for this pair ----
                    pvs = [pvpool.tile([P, QT], F32, tag="pv", name=f"pv_{pr}_{h2}")
                           for h2 in range(2)]
                    for gi in range(KT // 2):
                        a = 2 * gi
                        for h2 in range(2):
                            h_idx = 2 * pr + h2
                            rows = slice(h2 * HD, h2 * HD + HD)
                            stp = stpool.tile([P, 1024], F32, tag="st",
                                              name=f"st_{pr}_{a}_{h2}")
                            for j in range(2):
                                kt = a + j
                                nc.tensor.matmul(
                                    stp[:, j * 512:(j + 1) * 512],
                                    kT_sb[rows, kt * P:(kt + 1) * P],
                                    qT_t[rows, pr, :], start=True, stop=True)
                            ptt = atp.tile([P, 2, QT], F8, tag="pt",
                                           name=f"pt_{pr}_{a}_{h2}")
                            nc.scalar.activation(
                                ptt[:],
                                stp[:].rearrange("p (c n) -> p c n", n=512),
                                AF.Exp, scale=0.125 / (WS * WS))
                            nc.tensor.matmul(
                                pvs[h2][:HD + 1, :],
                                v_aug[:, a:a + 2, h_idx * (HD + 1):(h_idx + 1) * (HD + 1)],
                                ptt[:],
                                start=(gi == 0), stop=(gi == KT // 2 - 1),
                                perf_mode=DR)
                    for h2 in range(2):
                        rows = slice(h2 * HD, h2 * HD + HD)
                        den = atd.tile([1, QT], F32, tag="den", name=f"den_{pr}_{h2}")
                        nc.vector.reciprocal(den[:], pvs[h2][HD:HD + 1, :])
                        denb = atd.tile([HD, QT], F32, tag="denb",
                                        name=f"denb_{pr}_{h2}")
                        nc.gpsimd.partition_broadcast(denb[:], den[:])
                        nc.vector.tensor_tensor(outSB[rows, pr, :], pvs[h2][:HD, :],
                                                denb[:], ALU.mult)

            # ================= Output proj + residual + LN1 ===========
            h_t = pers.tile([P, TT, D], F32, tag="h")
            hT_t = pers.tile([P, DC, QT], F8, tag="hT")
            with tc.tile_pool(name="op", bufs=1) as op, \
                 tc.tile_pool(name="lnw", bufs=1) as lnw, \
                 tc.tile_pool(name="lnp3", bufs=2) as lnp3, \
                 tc.tile_pool(name="ops", bufs=1, space="PSUM") as ops, \
                 tc.tile_pool(name="tps", bufs=2, space="PSUM") as tps:
                if affine:
                    g1b_t = lnw.tile([P, D], F32, tag="g1b")
                    be1b_t = lnw.tile([P, D], F32, tag="be1b")
                    nc.scalar.dma_start(g1b_t[:], g1b[:])
                    nc.scalar.dma_start(be1b_t[:], be1b[:])
                else:
                    g1b_t = be1b_t = None
                wo_t = op.tile([P, NP2, 2 * D], F8, tag="wof")
                nc.scalar.dma_start(wo_t[:], wo8.rearrange("p (j m) -> p j m", j=NP2))
                xq_s = op.tile([P, TT, D], F32, tag="xqs")
                nc.sync.dma_start(xq_s[:], xq.rearrange("(t p) d -> p t d", p=P))
                for half in range(2):
                    tts = (2 * half, 2 * half + 1)
                    pso = [ops.tile([P, 512], F32, tag=f"ao{i}",
                                    name=f"ao_{half}_{i}") for i in range(4)]
                    for j in range(NP2):
                        for i, tt in enumerate(tts):
                            for oc in range(2):
                                nc.tensor.matmul(
                                    pso[i * 2 + oc],
                                    outSB[:, 2 * j:2 * j + 2, tt * P:(tt + 1) * P],
                                    wo_t[:, j, :].rearrange(
                                        "p (i o) -> p i o", i=2)[:, :, oc * 512:(oc + 1) * 512],
                                    start=(j == 0), stop=(j == NP2 - 1),
                                    perf_mode=DR)
                    for i, tt in enumerate(tts):
                        for oc in range(2):
                            nc.vector.tensor_tensor(
                                h_t[:, tt, oc * 512:(oc + 1) * 512],
                                pso[i * 2 + oc],
                                xq_s[:, tt, oc * 512:(oc + 1) * 512], ALU.add)
                        _layernorm(nc, lnp3, h_t[:, tt, :], h_t[:, tt, :],
                                   g1b_t, be1b_t, affine)
                        for dcg in range(2):
                            pst = tps.tile([P, 4, P], F32, tag="tp",
                                           name=f"tp_{tt}_{dcg}")
                            for k in range(4):
                                dc = 4 * dcg + k
                                nc.tensor.transpose(
                                    pst[:, k, :], h_t[:, tt, dc * P:(dc + 1) * P],
                                    ident[:])
                            nc.vector.tensor_copy(
                                hT_t[:, 4 * dcg:4 * dcg + 4, tt * P:(tt + 1) * P],
                                pst[:])

            # ================= FFN + LN2 ==============================
            ff1 = pers.tile([P, FC, QT], F8, tag="ff1")
            with tc.tile_pool(name="fp", bufs=2) as fp, \
                 tc.tile_pool(name="fw", bufs=1) as fw, \
                 tc.tile_pool(name="ft", bufs=2) as ft, \
                 tc.tile_pool(name="lnp4", bufs=2) as lnp4:
                with tc.tile_pool(name="f1s", bufs=4, space="PSUM") as f1s:
                    for fc4 in range(FC // 4):
                        w1_t = fp.tile([P, 4, DC2, 2 * P], F8, tag="wstream")
                        eng = nc.scalar if fc4 % 2 else nc.sync
                        eng.dma_start(w1_t[:], w18[:, fc4, :].rearrange(
                            "p (f c m) -> p f c m", f=4, c=DC2))
                        for f in range(4):
                            fc = 4 * fc4 + f
                            ps = f1s.tile([P, 512], F32, tag="mm")
                            for d2 in range(DC2):
                                nc.tensor.matmul(
                                    ps[:],
                                    w1_t[:, f, d2, :].rearrange("p (i m) -> p i m", i=2),
                                    hT_t[:, 2 * d2:2 * d2 + 2, :],
                                    start=(d2 == 0), stop=(d2 == DC2 - 1),
                                    perf_mode=DR)
                            # fused bias + relu (x16 scale kept; /256 in fc2)
                            nc.vector.tensor_scalar(ff1[:, fc, :], ps[:],
                                                    b1_t[:, fc:fc + 1], 0.0,
                                                    ALU.add, ALU.max)

                b2b_t = fw.tile([P, D], F32, tag="b2b")
                nc.scalar.dma_start(b2b_t[:], b2b[:])
                # fold the fc2 bias into the residual while ff1 runs
                for tt in range(TT):
                    nc.vector.tensor_tensor(h_t[:, tt, :], h_t[:, tt, :],
                                            b2b_t[:], ALU.add)
                if affine:
                    g2b_t = fw.tile([P, D], F32, tag="g2b")
                    be2b_t = fw.tile([P, D], F32, tag="be2b")
                    nc.scalar.dma_start(g2b_t[:], g2b[:])
                    nc.scalar.dma_start(be2b_t[:], be2b[:])
                else:
                    g2b_t = be2b_t = None
                with tc.tile_pool(name="f2s", bufs=1, space="PSUM") as f2s:
                    pss = [f2s.tile([P, 512], F32, tag=f"ff2_{i}", name=f"ff2_{i}")
                           for i in range(8)]
                    for s in range(FC2 // 2):
                        w2_t = fp.tile([P, 2, 2, D], F8, tag="wstream")
                        eng = nc.scalar if s % 2 else nc.sync
                        eng.dma_start(w2_t[:], w28[:, s, :].rearrange(
                            "p (g i o) -> p g i o", g=2, i=2))
                        for g in range(2):
                            f2 = 2 * s + g
                            for tt in range(TT):
                                for oc in range(2):
                                    nc.tensor.matmul(
                                        pss[tt * 2 + oc],
                                        ff1[:, 2 * f2:2 * f2 + 2, tt * P:(tt + 1) * P],
                                        w2_t[:, g, :, oc * 512:(oc + 1) * 512],
                                        start=(f2 == 0), stop=(f2 == FC2 - 1),
                                        perf_mode=DR)
                    for tt in range(TT):
                        t2 = ft.tile([P, D], F32, tag="t2")
                        for oc in range(2):
                            # (psum / 256) + h  (undo w1*16 * w2*16)
                            nc.vector.scalar_tensor_tensor(
                                t2[:, oc * 512:(oc + 1) * 512],
                                pss[tt * 2 + oc], 1.0 / (WS * WS),
                                h_t[:, tt, oc * 512:(oc + 1) * 512],
                                ALU.mult, ALU.add)
                        _layernorm(nc, lnp4, t2[:], t2[:], g2b_t, be2b_t, affine)
                        nc.sync.dma_start(
                            y.rearrange("(t p) d -> p t d", p=P)[:, tt, :], t2[:])

    nc.compile()
    return nc


def _layernorm(nc, pool, dst, src, g_t, be_t, affine):
    """dst = (src - mean)/sqrt(var + eps) [* g + be], row-wise over 1024.

    bn_stats/bn_aggr produce mean+var in one DVE pass. rsqrt is computed
    as exp(-0.5*ln(v)) on ACT (both funcs live in one activation table,
    so no table thrash with the attention exps) and refined with one
    Newton step on DVE.
    """
    stats = pool.tile([P, 2, 6], F32, tag="ln_st")
    nc.vector.bn_stats(stats[:, 0, :], src[:, 0:D // 2])
    nc.vector.bn_stats(stats[:, 1, :], src[:, D // 2:D])
    mv = pool.tile([P, 2], F32, tag="ln_mv")
    nc.vector.bn_aggr(mv[:], stats[:])
    vv = pool.tile([P, 1], F32, tag="ln_v")
    nc.vector.tensor_scalar(vv[:], mv[:, 1:2], EPS, None, ALU.add)
    lnv = pool.tile([P, 1], F32, tag="ln_ln")
    nc.scalar.activation(lnv[:], vv[:], AF.Ln)
    r = pool.tile([P, 1], F32, tag="ln_r")
    nc.scalar.activation(r[:], lnv[:], AF.Exp, scale=-0.5)
    # one Newton step: r <- r * (1.5 - 0.5 * vv * r^2)
    t = pool.tile([P, 1], F32, tag="ln_t")
    nc.vector.tensor_tensor(t[:], r[:], r[:], ALU.mult)
    nc.vector.tensor_tensor(t[:], t[:], vv[:], ALU.mult)
    nc.vector.tensor_scalar(t[:], t[:], -0.5, 1.5, ALU.mult, ALU.add)
    nc.vector.tensor_tensor(r[:], r[:], t[:], ALU.mult)
    nc.vector.tensor_scalar(dst, src, mv[:, 0:1], r[:], ALU.subtract, ALU.mult)
    if affine:
        nc.vector.tensor_tensor(dst, dst, g_t[:], ALU.mult)
        nc.vector.tensor_tensor(dst, dst, be_t[:], ALU.add)


def make_in_maps(x, w_qkv, b_qkv, w_o, b_o, g1, be1, w1, b1, w2, b2, g2, be2):
    f = np.float32
    f8 = mybir.dt.np(F8)
    x = np.asarray(x, f)
    w_qkv = np.asarray(w_qkv, f)
    b_qkv = np.asarray(b_qkv, f)
    bc = lambda v: np.ascontiguousarray(
        np.broadcast_to(np.asarray(v, f).reshape(1, D), (P, D)))

    # [d, m] -> [p, pr, (d2 i m)] partition-major DoubleRow layout
    def qk_layout(w):
        return np.ascontiguousarray(
            (w * WS).reshape(DC2, 2, P, NPAIR, P).transpose(2, 3, 0, 1, 4)
            .reshape(P, NPAIR, 2 * DC2 * P)).astype(f8)

    wv_h = np.ascontiguousarray(
        (w_qkv[:, 2 * D:] * WS).reshape(DC2, 2, P, 2, 512)
        .transpose(2, 3, 0, 1, 4).reshape(P, 2 * DC2 * 2 * 512)).astype(f8)
    wo_h = np.ascontiguousarray(
        np.asarray(w_o, f).reshape(NP2, 2, P, D).transpose(2, 0, 1, 3)
        .reshape(P, NP2 * 2 * D)).astype(f8)
    w1_h = np.ascontiguousarray(
        (np.asarray(w1, f) * WS).reshape(DC2, 2, P, FC // 4, 4, P)
        .transpose(2, 3, 4, 0, 1, 5).reshape(P, FC // 4, 4 * DC2 * 2 * P)).astype(f8)
    w2_h = np.ascontiguousarray(
        (np.asarray(w2, f) * WS).reshape(FC2 // 2, 2, 2, P, D)
        .transpose(3, 0, 1, 2, 4).reshape(P, FC2 // 2, 2 * 2 * D)).astype(f8)

    shared = {
        "wq8": qk_layout(w_qkv[:, :D]),
        "wk8": qk_layout(w_qkv[:, D:2 * D]),
        "wv8": wv_h, "wo8": wo_h, "w18": w1_h, "w28": w2_h,
        "bq": np.ascontiguousarray((b_qkv[:D] * WS).reshape(NPAIR, P).T),
        "bk": np.ascontiguousarray((b_qkv[D:2 * D] * WS).reshape(NPAIR, P).T),
        "b1": np.ascontiguousarray((np.asarray(b1, f) * WS).reshape(FC, P).T),
        "bvb": bc(np.asarray(b_qkv[2 * D:], f) * WS), "b2b": bc(b2),
        "g1b": bc(g1), "be1b": bc(be1), "g2b": bc(g2), "be2b": bc(be2),
        "vones": np.full((P, KT), WS, f).astype(f8),
    }
    in_maps = []
    for c in range(8):
        n, qi = divmod(c, 4)
        xT8n = np.ascontiguousarray(x[n].T).astype(f8)
        m = dict(shared)
        m["xT8"] = xT8n
        m["xTq8"] = np.ascontiguousarray(xT8n[:, qi * QT:(qi + 1) * QT])
        m["xq"] = np.ascontiguousarray(x[n, qi * QT:(qi + 1) * QT, :]
                                       + np.asarray(b_o, f).reshape(1, D))
        in_maps.append(m)
    return in_maps


def get_nc(affine=True):
    if affine not in _CACHED_NC:
        _CACHED_NC[affine] = _build_nc(affine)
    return _CACHED_NC[affine]


def kernel(**inputs):
    in_maps = make_in_maps(**inputs)
    affine = not (np.all(np.asarray(inputs["g1"]) == 1)
                  and np.all(np.asarray(inputs["be1"]) == 0)
                  and np.all(np.asarray(inputs["g2"]) == 1)
                  and np.all(np.asarray(inputs["be2"]) == 0))
    nc = get_nc(affine)
    # The axon-proxied NRT occasionally reports a transient
    # NRT_EXEC_UNIT_UNRECOVERABLE on a cold first dispatch; a plain retry
    # has always succeeded with bit-identical results, so recover inline.
    last_err = None
    for _ in range(3):
        try:
            res = run_bass_kernel_spmd(nc, in_maps, list(range(8))).results
            break
        except Exception as e:  # noqa: BLE001
            last_err = e
    else:
        raise last_err
    yout = np.empty((NB, L, D), np.float32)
    for c in range(8):
        n, qi = divmod(c, 4)
        yout[n, qi * QT:(qi + 1) * QT] = res[c]["y"]
    return yout


if __name__ == "__main__":
    rng = np.random.default_rng(0)
    demo = {
        "x": rng.standard_normal((NB, L, D)).astype(np.float32),
        "w_qkv": rng.standard_normal((D, 3 * D)).astype(np.float32) * 0.03,
        "b_qkv": rng.standard_normal(3 * D).astype(np.float32) * 0.03,
        "w_o": rng.standard_normal((D, D)).astype(np.float32) * 0.03,
        "b_o": rng.standard_normal(D).astype(np.float32) * 0.03,
        "g1": np.ones(D, np.float32), "be1": np.zeros(D, np.float32),
        "w1": rng.standard_normal((D, FF)).astype(np.float32) * 0.03,
        "b1": rng.standard_normal(FF).astype(np.float32) * 0.03,
        "w2": rng.standard_normal((FF, D)).astype(np.float32) * 0.015,
        "b2": rng.standard_normal(D).astype(np.float32) * 0.015,
        "g2": np.ones(D, np.float32), "be2": np.zeros(D, np.float32),
    }
    out = kernel(**demo)
    print("kernel output:", out.shape, out.dtype, np.abs(out).mean())


# revision 20
# speedup vs baseline: 2.0288x; 1.2722x over previous
"""Trainium2 Bass kernel for a transformer encoder layer (nn_Encoder).

x:[2,2048,1024] f32. 8 NeuronCores, data-parallel: core c handles batch
n=c//4, query rows qi=c%4 (512 tokens). K/V are recomputed per core for
the full batch (x4 redundancy) to avoid collectives (~300us for the
8.4MB all-reduce this would replace).

All matmuls are fp8 e4m3 DoubleRow (2x PE column rate, 256-deep
contraction per instruction). K/Q are produced in a split-hd layout
([32 partitions, 2 k-tiles] per head) so even the hd=64 score matmuls
run DoubleRow. Weights are host-scaled x16 into the fp8 normal range;
the scale unwinds via the softmax ones-row (=16), the exp scale
(0.125/256) and a /256 on the fc2 PSUM. Residuals/LayerNorm stay f32.

The softmax exp stream on the Activation engine (~133us) is the
critical resource. Attention is split into two query-halves: while
half 1's exps run, half 0's output-proj/LN1/FFN execute in the PE/DVE
shadow (emitted as interleaved filler chunks); K/V projections for
later head-quads fill the shadow of half 0.
"""
import os
import sys
from contextlib import ExitStack

for _p in ("/opt/trn_rl_repo", "/root/.axon_site/_ro/trn_rl_repo"):
    if os.path.isdir(_p) and _p not in sys.path:
        sys.path.insert(0, _p)

import numpy as np
import concourse.bass as bass
import concourse.mybir as mybir
import concourse.tile as tile
from concourse import bacc
from concourse.bass_utils import run_bass_kernel_spmd
from concourse.masks import make_identity

F32 = mybir.dt.float32
F8 = mybir.dt.float8e4
AF = mybir.ActivationFunctionType
ALU = mybir.AluOpType
DR = mybir.MatmulPerfMode.DoubleRow

D = 1024
H = 16
HD = 64
FF = 4096
L = 2048
NB = 2
P = 128
QT = 512          # query tokens per core
QH = QT // 2      # query half
DC = D // P       # 8 chunks of the model dim
DC2 = DC // 2     # 4 DoubleRow chunk-pairs
KT = L // P       # 16 key tiles
FC = FF // P      # 32 ff chunks
FC2 = FC // 2     # 16 ff chunk-pairs
TT = QT // P      # 4 own-token tiles
NPAIR = H // 2    # 8 head pairs
NP2 = NPAIR // 2  # 4 pair-pairs
NQ = H // 4       # 4 head quads (scores split layout)
EPS = 1e-5
WS = 16.0         # host weight scale (fp8 range)

_CACHED_NC = {}


def _build_nc(affine=True):
    nc = bacc.Bacc("TRN2", target_bir_lowering=False)

    def dparam(name, shape, dt=F8):
        return nc.dram_tensor(name, shape, dt, kind="ExternalInput")

    xT8 = dparam("xT8", [D, L])            # x[n].T, fp8
    xTq8 = dparam("xTq8", [D, QT])         # own-token columns of xT, fp8
    xq = dparam("xq", [QT, D], F32)        # own tokens, natural (residual)
    # weights: partition-major fp8, DoubleRow k-tile-pair layouts
    wqs = dparam("wqs", [P, 2 * NQ, 2 * DC2 * P])    # [p][(g i)][d2 ik m]
    wks = dparam("wks", [P, 2 * NQ, 2 * DC2 * P])
    wv8 = dparam("wv8", [P, 2 * DC2 * 2 * 512])      # [p][vh d2 ik n]
    wo8 = dparam("wo8", [P, NP2 * 2 * D])            # [p][j ik o]
    w18 = dparam("w18", [P, FC // 4, 4 * (DC2 + 1) * 2 * P])  # [p][f4][f d2 ik m]
    w28 = dparam("w28", [P, FC2 // 2, 2 * 2 * D])        # [p][s][g ik o]
    bqs = dparam("bqs", [P, 2 * NQ], F32)  # x16 biases, split-hd order
    bks = dparam("bks", [P, 2 * NQ], F32)
    b1 = dparam("b1", [P, FC], F32)
    bvb = dparam("bvb", [P, D], F32)
    b2b = dparam("b2b", [P, D], F32)       # natural scale
    g1b = dparam("g1b", [P, D], F32)
    be1b = dparam("be1b", [P, D], F32)
    g2b = dparam("g2b", [P, D], F32)
    be2b = dparam("be2b", [P, D], F32)
    vones = dparam("vones", [P, KT])       # value 16 (denominator row)
    hc8 = dparam("hc8", [P, 2 * QT])       # F1 bias rows: [16,0...;0...]

    y = nc.dram_tensor("y", [QT, D], F32, kind="ExternalOutput")

    with tile.TileContext(nc) as tc, ExitStack() as es:
        # Pre-load the one activation table that serves every ACT func we
        # use (Exp for softmax, Ln+Exp for the LN rsqrt): without this the
        # first-fit chooser thrashes exp<->ln tables at 1283ns per load.
        from concourse.hw_specs import get_activation_tables
        _tabs = get_activation_tables(nc.m.arch)
        _idx = next(i for i, (_, s) in enumerate(_tabs.items())
                    if AF.Exp in s and AF.Ln in s)
        nc.scalar.add_instruction(mybir.InstLoadActFuncSet(
            name=nc.scalar.bass.get_next_instruction_name(),
            act_func_set_id=_idx, ins=[], outs=[]))

        pers = es.enter_context(tc.tile_pool(name="pers", bufs=1))
        ident = pers.tile([P, P], F32, tag="ident")
        make_identity(nc, ident[:])
        bqs_t = pers.tile([P, 2 * NQ], F32, tag="bqs")
        bks_t = pers.tile([P, 2 * NQ], F32, tag="bks")
        b1_t = pers.tile([P, FC], F32, tag="b1")
        bvb_t = pers.tile([P, D], F32, tag="bvb")
        # b2b reuses bvb's slot: bvb is dead after V-proj, long before
        # the LNT chunks fold b2 into the residual.
        b2b_t = pers.tile([P, D], F32, tag="bvb", name="b2b_t")

        xT_t = pers.tile([P, DC, L], F8, tag="xT")
        xTq_t = pers.tile([P, DC, QT], F8, tag="xTq")
        v_aug = pers.tile([P, KT, H * (HD + 1)], F8, tag="vaug")
        ones_t = pers.tile([P, KT], F8, tag="ones")
        qT_all = pers.tile([P, NQ, 2, QT], F8, tag="qT")
        kT_all = pers.tile([P, NQ, 2, L], F8, tag="kT")
        outSB = pers.tile([P, NPAIR, QT], F8, tag="outSB")
        h_t = pers.tile([P, TT, D], F32, tag="h")
        hT_t = pers.tile([P, DC + 2, QT], F8, tag="hT")
        ff1 = pers.tile([P, FC, QT], F8, tag="ff1")
        xq_s = pers.tile([P, TT, D], F32, tag="xqs")
        wqs_t = pers.tile([P, 2 * NQ, DC2, 2 * P], F8, tag="wqs")
        wks_t = pers.tile([P, 2 * NQ, DC2, 2 * P], F8, tag="wks")
        wv_t = pers.tile([P, 2, DC2, 2 * 512], F8, tag="wv")
        wo_t = pers.tile([P, NP2, 2 * D], F8, tag="wof")
        w2_t = pers.tile([P, FC2 // 2, 2, 2 * D], F8, tag="w2")
        if affine:
            g1b_t = pers.tile([P, D], F32, tag="g1b")
            be1b_t = pers.tile([P, D], F32, tag="be1b")
            g2b_t = pers.tile([P, D], F32, tag="g2b")
            be2b_t = pers.tile([P, D], F32, tag="be2b")
        else:
            g1b_t = be1b_t = g2b_t = be2b_t = None

        # startup DMAs (sync queue; keep ACT's sequencer free for exps)
        nc.sync.dma_start(xTq_t[:], xTq8.rearrange("(c p) t -> p c t", p=P))
        nc.scalar.dma_start(bqs_t[:], bqs[:])
        nc.scalar.dma_start(bks_t[:], bks[:])
        nc.scalar.dma_start(b1_t[:], b1[:])
        nc.scalar.dma_start(bvb_t[:], bvb[:])
        nc.sync.dma_start(
            wqs_t[:], wqs.rearrange("p b (c m) -> p b c m", c=DC2))
        nc.sync.dma_start(
            wks_t[:], wks.rearrange("p b (c m) -> p b c m", c=DC2))
        nc.sync.dma_start(
            wv_t[:], wv8.rearrange("p (v c m) -> p v c m", v=2, c=DC2))
        nc.scalar.dma_start(ones_t[:], vones[:])
        nc.scalar.dma_start(
            hT_t[:, DC:DC + 2, :],
            hc8.rearrange("p (k t) -> p k t", k=2))

        for dc in range(DC):
            nc.sync.dma_start(
                xT_t[:, dc, :],
                xT8.rearrange("(c p) t -> p c t", p=P)[:, dc, :])
        nc.vector.tensor_copy(
            v_aug[:].rearrange("p t (h c) -> p t h c", c=HD + 1)[:, :, :, HD],
            ones_t[:, :, None].to_broadcast([P, KT, H]))
        # mid-kernel loads, all needed only by the post-attention chunks
        nc.sync.dma_start(xq_s[:], xq.rearrange("(t p) d -> p t d", p=P))
        nc.sync.dma_start(wo_t[:], wo8.rearrange("p (j m) -> p j m", j=NP2))
        nc.sync.dma_start(w2_t[:], w28.rearrange("p s (g m) -> p s g m", g=2))
        if affine:
            nc.scalar.dma_start(g1b_t[:], g1b[:])
            nc.scalar.dma_start(be1b_t[:], be1b[:])
            nc.scalar.dma_start(g2b_t[:], g2b[:])
            nc.scalar.dma_start(be2b_t[:], be2b[:])

        stp = es.enter_context(tc.tile_pool(name="stp", bufs=2, space="PSUM"))
        pvp = es.enter_context(tc.tile_pool(name="pvp", bufs=2, space="PSUM"))
        ppp = es.enter_context(tc.tile_pool(name="ppp", bufs=2))
        atd = es.enter_context(tc.tile_pool(name="atd", bufs=1))
        lnp = es.enter_context(tc.tile_pool(name="lnp", bufs=2))
        fp = es.enter_context(tc.tile_pool(name="fp", bufs=2))
        ft = es.enter_context(tc.tile_pool(name="ft", bufs=2))
        kvp_es = ExitStack()
        kvp = kvp_es.enter_context(tc.tile_pool(name="kvp", bufs=2,
                                                space="PSUM"))

        def emit_qproj(g):
            for i in range(2):
                b = 2 * g + i
                ps = kvp.tile([P, 512], F32, tag="kv", name=f"qps_{b}")
                for d2 in range(DC2):
                    nc.tensor.matmul(
                        ps[:],
                        wqs_t[:, b, d2, :].rearrange("p (i m) -> p i m", i=2),
                        xTq_t[:, 2 * d2:2 * d2 + 2, :],
                        start=(d2 == 0), stop=(d2 == DC2 - 1), perf_mode=DR)
                nc.vector.tensor_scalar(
                    qT_all[:, g, i, :], ps[:],
                    bqs_t[:, b:b + 1], None, ALU.add)

        def emit_kproj(g, i):
            """kT_all[:, g, i, :] for one hd-half of head-quad g."""
            b = 2 * g + i
            for blk in range(4):     # 512-key blocks
                ps = kvp.tile([P, 512], F32, tag="kv",
                              name=f"kps_{g}_{i}_{blk}")
                for d2 in range(DC2):
                    nc.tensor.matmul(
                        ps[:],
                        wks_t[:, b, d2, :].rearrange("p (i m) -> p i m", i=2),
                        xT_t[:, 2 * d2:2 * d2 + 2,
                             blk * 512:(blk + 1) * 512],
                        start=(d2 == 0), stop=(d2 == DC2 - 1), perf_mode=DR)
                nc.vector.tensor_scalar(
                    kT_all[:, g, i, blk * 512:(blk + 1) * 512], ps[:],
                    bks_t[:, b:b + 1], None, ALU.add)

        def emit_vsub(vh, sub):
            """v_aug columns for v-half vh, key tiles 4*sub..4*sub+3."""
            for tt in range(4 * sub, 4 * sub + 4):
                ps = kvp.tile([P, 512], F32, tag="kv",
                              name=f"vps_{vh}_{tt}")
                for d2 in range(DC2):
                    nc.tensor.matmul(
                        ps[:],
                        xT_t[:, 2 * d2:2 * d2 + 2, tt * P:(tt + 1) * P],
                        wv_t[:, vh, d2, :].rearrange("p (i n) -> p i n", i=2),
                        start=(d2 == 0), stop=(d2 == DC2 - 1), perf_mode=DR)
                dst = v_aug[:, tt, :].rearrange(
                    "p (h c) -> p h c", c=HD + 1)[:, vh * 8:(vh + 1) * 8, 0:HD]
                nc.vector.tensor_tensor(
                    dst, ps[:].rearrange("p (h c) -> p h c", c=HD),
                    bvb_t[:, vh * 512:(vh + 1) * 512].rearrange(
                        "p (h c) -> p h c", c=HD),
                    ALU.add)

        emit_qproj(0)
        emit_kproj(0, 0)
        emit_kproj(0, 1)
        emit_vsub(0, 0)

        fillers = []

        def drain():
            if fillers:
                fillers.pop(0)()

        def emit_attn(qh, pr):
            g, j0 = pr // 2, 2 * (pr % 2)
            pvs = [pvp.tile([P, 512], F32, tag="pv", name=f"pv_{qh}_{pr}_{h2}")
                   for h2 in range(2)]
            for grp in range(KT // 2):
                st = stp.tile([P, 2, 2, 256], F32, tag="st",
                              name=f"st_{qh}_{pr}_{grp}")
                for h2 in range(2):
                    j = j0 + h2
                    rows = slice(32 * j, 32 * j + 32)
                    for k in range(2):
                        kt = 2 * grp + k
                        nc.tensor.matmul(
                            st[:, h2, k, :],
                            kT_all[rows, g, :, kt * P:(kt + 1) * P],
                            qT_all[rows, g, :, qh * QH:(qh + 1) * QH],
                            start=True, stop=True, perf_mode=DR,
                            tile_position=(32 * j, 0))
                pp = ppp.tile([P, 2, 2, 256], F8, tag="pp",
                              name=f"pp_{qh}_{pr}_{grp}")
                nc.scalar.activation(pp[:], st[:], AF.Exp,
                                     scale=0.125 / (WS * WS))
                for h2 in range(2):
                    h = 2 * pr + h2
                    nc.tensor.matmul(
                        pvs[h2][:HD + 1, :QH],
                        v_aug[:, 2 * grp:2 * grp + 2,
                              h * (HD + 1):(h + 1) * (HD + 1)],
                        pp[:, h2, :, :],
                        start=(grp == 0), stop=(grp == KT // 2 - 1),
                        perf_mode=DR)
                if grp in (0, 2, 5):
                    drain()
            for h2 in range(2):
                rows = slice(h2 * HD, h2 * HD + HD)
                den = atd.tile([1, QH], F32, tag="den",
                               name=f"den_{qh}_{pr}_{h2}")
                nc.vector.reciprocal(den[:], pvs[h2][HD:HD + 1, :QH])
                denb = atd.tile([HD, QH], F32, tag="denb",
                                name=f"denb_{qh}_{pr}_{h2}")
                nc.gpsimd.partition_broadcast(denb[:], den[:])
                nc.vector.tensor_tensor(
                    outSB[rows, pr, qh * QH:(qh + 1) * QH],
                    pvs[h2][:HD, :QH], denb[:], ALU.mult)

        # ---------- post-attention chunk emitters (token-tile tg) ----------
        postp_es = ExitStack()
        postp = [None]

        def emit_O(qh, tt):
            tg = 2 * qh + tt
            for oc in range(2):
                ps = postp[0].tile([P, 512], F32, tag="post",
                                   name=f"ops_{tg}_{oc}")
                for j2 in range(NP2):
                    nc.tensor.matmul(
                        ps[:],
                        outSB[:, 2 * j2:2 * j2 + 2, tg * P:(tg + 1) * P],
                        wo_t[:, j2, :].rearrange(
                            "p (i o) -> p i o", i=2)[:, :, oc * 512:(oc + 1) * 512],
                        start=(j2 == 0), stop=(j2 == NP2 - 1), perf_mode=DR)
                nc.vector.tensor_tensor(
                    h_t[:, tg, oc * 512:(oc + 1) * 512], ps[:],
                    xq_s[:, tg, oc * 512:(oc + 1) * 512], ALU.add)

        def emit_LNT(qh, tt):
            tg = 2 * qh + tt
            _layernorm(nc, lnp, h_t[:, tg, :], h_t[:, tg, :],
                       g1b_t, be1b_t, affine)
            for dcg in range(2):
                tp = postp[0].tile([P, 512], F32, tag="post",
                                   name=f"tp_{tg}_{dcg}")
                for k in range(4):
                    dc = 4 * dcg + k
                    nc.tensor.transpose(
                        tp[:, k * P:(k + 1) * P],
                        h_t[:, tg, dc * P:(dc + 1) * P], ident[:])
                nc.vector.tensor_copy(
                    hT_t[:, 4 * dcg:4 * dcg + 4, tg * P:(tg + 1) * P],
                    tp[:].rearrange("p (k m) -> p k m", k=4))
            # fold the fc2 bias into the residual (after transposes read h)
            nc.vector.tensor_tensor(h_t[:, tg, :], h_t[:, tg, :],
                                    b2b_t[:], ALU.add)

        def emit_F1(qh, fcg, use_act=False):
            w1_t = fp.tile([P, 4, DC2 + 1, 2 * P], F8, tag="w1s")
            nc.sync.dma_start(w1_t[:], w18[:, fcg, :].rearrange(
                "p (f c m) -> p f c m", f=4, c=DC2 + 1))
            for u in range(2):
                ps = postp[0].tile([P, 512], F32, tag="post",
                                   name=f"f1_{qh}_{fcg}_{u}")
                for f in (2 * u, 2 * u + 1):
                    fc = 4 * fcg + f
                    for d2 in range(DC2 + 1):
                        nc.tensor.matmul(
                            ps[:, (f - 2 * u) * QH:(f - 2 * u + 1) * QH],
                            w1_t[:, f, d2, :].rearrange(
                                "p (i m) -> p i m", i=2),
                            hT_t[:, 2 * d2:2 * d2 + 2, qh * QH:(qh + 1) * QH],
                            start=(d2 == 0), stop=(d2 == DC2), perf_mode=DR,
                            skip_group_check=True)
                dst = ff1[:, 4 * fcg + 2 * u:4 * fcg + 2 * u + 2,
                          qh * QH:(qh + 1) * QH]
                src_ap = ps[:].rearrange("p (f n) -> p f n", f=2)
                if use_act and u == 1:
                    nc.scalar.activation(dst, src_ap, AF.Relu)
                else:
                    nc.vector.tensor_scalar(dst, src_ap, 0.0, None, ALU.max)

        def emit_F2(qh, tt, oc):
            tg = 2 * qh + tt
            if oc == 0:
                _f2_t2[tg] = ft.tile([P, D], F32, tag="t2", name=f"t2_{tg}")
            ps = postp[0].tile([P, 512], F32, tag="post",
                               name=f"f2_{tg}_{oc}")
            for f2 in range(FC2):
                s, g2 = f2 // 2, f2 % 2
                nc.tensor.matmul(
                    ps[:],
                    ff1[:, 2 * f2:2 * f2 + 2, tg * P:(tg + 1) * P],
                    w2_t[:, s, g2, :].rearrange(
                        "p (i o) -> p i o", i=2)[:, :, oc * 512:(oc + 1) * 512],
                    start=(f2 == 0), stop=(f2 == FC2 - 1), perf_mode=DR)
            t2 = _f2_t2[tg]
            nc.vector.scalar_tensor_tensor(
                t2[:, oc * 512:(oc + 1) * 512], ps[:], 1.0 / (WS * WS),
                h_t[:, tg, oc * 512:(oc + 1) * 512], ALU.mult, ALU.add)
            if oc == 1:
                _layernorm(nc, lnp, t2[:], t2[:], g2b_t, be2b_t, affine)
                nc.sync.dma_start(
                    y.rearrange("(t p) d -> p t d", p=P)[:, tg, :], t2[:])

        _f2_t2 = {}

        # ---------------- schedule ----------------
        fillers.extend([
            lambda: emit_vsub(0, 1), lambda: emit_vsub(0, 2),
            lambda: emit_vsub(0, 3),
            lambda: emit_qproj(1),
            lambda: emit_kproj(1, 0), lambda: emit_kproj(1, 1),
            lambda: emit_vsub(1, 0), lambda: emit_vsub(1, 1),
            lambda: emit_qproj(2),
            lambda: emit_kproj(2, 0), lambda: emit_kproj(2, 1),
            lambda: emit_vsub(1, 2), lambda: emit_vsub(1, 3),
            lambda: emit_qproj(3),
            lambda: emit_kproj(3, 0), lambda: emit_kproj(3, 1),
        ])
        for pr in range(NPAIR):
            emit_attn(0, pr)
        while fillers:
            drain()
        kvp_es.close()
        nc.scalar.dma_start(b2b_t[:], b2b[:])
        postp[0] = postp_es.enter_context(
            tc.tile_pool(name="postp", bufs=2, space="PSUM"))
        fillers.extend(
            [lambda tt=tt: emit_O(0, tt) for tt in range(2)] +
            [lambda tt=tt: emit_LNT(0, tt) for tt in range(2)] +
            [lambda f=f: emit_F1(0, f) for f in range(FC // 4)] +
            [lambda tt=tt, oc=oc: emit_F2(0, tt, oc)
             for tt in range(2) for oc in range(2)])
        for pr in range(NPAIR):
            emit_attn(1, pr)
        while fillers:
            drain()
        for tt in range(2):
            emit_O(1, tt)
            emit_LNT(1, tt)
        for f in range(FC // 4):
            emit_F1(1, f, use_act=True)
        for tt in range(2):
            for oc in range(2):
                emit_F2(1, tt, oc)
        postp_es.close()

    nc.compile()
    return nc


def _layernorm(nc, pool, dst, src, g_t, be_t, affine):
    """dst = (src - mean)/sqrt(var + eps) [* g + be], row-wise over 1024.

    bn_stats/bn_aggr produce mean+var in one DVE pass. rsqrt is computed
    as exp(-0.5*ln(v)) on ACT (both funcs live in one activation table,
    so no table thrash with the attention exps) and refined with one
    Newton step on DVE.
    """
    stats = pool.tile([P, 2, 6], F32, tag="ln_st")
    nc.vector.bn_stats(stats[:, 0, :], src[:, 0:D // 2])
    nc.vector.bn_stats(stats[:, 1, :], src[:, D // 2:D])
    mv = pool.tile([P, 2], F32, tag="ln_mv")
    nc.vector.bn_aggr(mv[:], stats[:])
    vv = pool.tile([P, 1], F32, tag="ln_v")
    nc.vector.tensor_scalar(vv[:], mv[:, 1:2], EPS, None, ALU.add)
    lnv = pool.tile([P, 1], F32, tag="ln_ln")
    nc.scalar.activation(lnv[:], vv[:], AF.Ln)
    r = pool.tile([P, 1], F32, tag="ln_r")
    nc.scalar.activation(r[:], lnv[:], AF.Exp, scale=-0.5)
    # one Newton step: r <- r * (1.5 - 0.5 * vv * r^2)
    t = pool.tile([P, 1], F32, tag="ln_t")
    nc.vector.tensor_tensor(t[:], r[:], r[:], ALU.mult)
    nc.vector.tensor_tensor(t[:], t[:], vv[:], ALU.mult)
    nc.vector.tensor_scalar(t[:], t[:], -0.5, 1.5, ALU.mult, ALU.add)
    nc.vector.tensor_tensor(r[:], r[:], t[:], ALU.mult)
    nc.vector.tensor_scalar(dst, src, mv[:, 0:1], r[:], ALU.subtract, ALU.mult)
    if affine:
        nc.vector.tensor_tensor(dst, dst, g_t[:], ALU.mult)
        nc.vector.tensor_tensor(dst, dst, be_t[:], ALU.add)


def _hc8():
    h = np.zeros((P, 2 * QT), np.float32)
    h[0, :QT] = WS
    return h.astype(mybir.dt.np(F8))


def make_in_maps(x, w_qkv, b_qkv, w_o, b_o, g1, be1, w1, b1, w2, b2, g2, be2):
    f = np.float32
    f8 = mybir.dt.np(F8)
    x = np.asarray(x, f)
    w_qkv = np.asarray(w_qkv, f)
    b_qkv = np.asarray(b_qkv, f)
    bc = lambda v: np.ascontiguousarray(
        np.broadcast_to(np.asarray(v, f).reshape(1, D), (P, D)))

    # [d, h*64+hd] -> [p][(g i)][d2 ik m] with m=32j+r -> (head 4g+j, hd 32i+r)
    def qk_split(w):
        t = (w * WS).reshape(DC2, 2, P, NQ, 4, 2, 32)
        t = t.transpose(2, 3, 5, 0, 1, 4, 6)    # [p, g, i, d2, ik, j, r]
        return np.ascontiguousarray(t.reshape(P, 2 * NQ, 2 * DC2 * P)).astype(f8)

    def bias_split(b):
        t = (b * WS).reshape(NQ, 4, 2, 32).transpose(1, 3, 0, 2)  # [j, r, g, i]
        return np.ascontiguousarray(t.reshape(P, 2 * NQ))

    wv_h = np.ascontiguousarray(
        (w_qkv[:, 2 * D:] * WS).reshape(DC2, 2, P, 2, 512)
        .transpose(2, 3, 0, 1, 4).reshape(P, 2 * DC2 * 2 * 512)).astype(f8)
    wo_h = np.ascontiguousarray(
        np.asarray(w_o, f).reshape(NP2, 2, P, D).transpose(2, 0, 1, 3)
        .reshape(P, NP2 * 2 * D)).astype(f8)
    w1_base = ((np.asarray(w1, f) * WS).reshape(DC2, 2, P, FC // 4, 4, P)
               .transpose(2, 3, 4, 0, 1, 5))          # [p, f4, f, d2, ik, m]
    w1_bias = np.zeros((P, FC // 4, 4, 1, 2, P), f)
    w1_bias[0, :, :, 0, 0, :] = np.asarray(b1, f).reshape(FC // 4, 4, P)
    w1_h = np.ascontiguousarray(
        np.concatenate([w1_base, w1_bias], axis=3)
        .reshape(P, FC // 4, 4 * (DC2 + 1) * 2 * P)).astype(f8)
    w2_h = np.ascontiguousarray(
        (np.asarray(w2, f) * WS).reshape(FC2 // 2, 2, 2, P, D)
        .transpose(3, 0, 1, 2, 4).reshape(P, FC2 // 2, 2 * 2 * D)).astype(f8)

    shared = {
        "wqs": qk_split(w_qkv[:, :D]),
        "wks": qk_split(w_qkv[:, D:2 * D]),
        "wv8": wv_h, "wo8": wo_h, "w18": w1_h, "w28": w2_h,
        "bqs": bias_split(b_qkv[:D]),
        "bks": bias_split(b_qkv[D:2 * D]),
        "b1": np.ascontiguousarray((np.asarray(b1, f) * WS).reshape(FC, P).T),
        "bvb": bc(np.asarray(b_qkv[2 * D:], f) * WS), "b2b": bc(b2),
        "g1b": bc(g1), "be1b": bc(be1), "g2b": bc(g2), "be2b": bc(be2),
        "vones": np.full((P, KT), WS, f).astype(f8),
        "hc8": _hc8(),
    }
    in_maps = []
    for c in range(8):
        n, qi = divmod(c, 4)
        xT8n = np.ascontiguousarray(x[n].T).astype(f8)
        m = dict(shared)
        m["xT8"] = xT8n
        m["xTq8"] = np.ascontiguousarray(xT8n[:, qi * QT:(qi + 1) * QT])
        m["xq"] = np.ascontiguousarray(x[n, qi * QT:(qi + 1) * QT, :]
                                       + np.asarray(b_o, f).reshape(1, D))
        in_maps.append(m)
    return in_maps


def get_nc(affine=True):
    if affine not in _CACHED_NC:
        _CACHED_NC[affine] = _build_nc(affine)
    return _CACHED_NC[affine]


def kernel(**inputs):
    in_maps = make_in_maps(**inputs)
    affine = not (np.all(np.asarray(inputs["g1"]) == 1)
                  and np.all(np.asarray(inputs["be1"]) == 0)
                  and np.all(np.asarray(inputs["g2"]) == 1)
                  and np.all(np.asarray(inputs["be2"]) == 0))
    nc = get_nc(affine)
    # The axon-proxied NRT occasionally reports a transient
    # NRT_EXEC_UNIT_UNRECOVERABLE on a cold first dispatch; a plain retry
    # has always succeeded with bit-identical results, so recover inline.
    last_err = None
    for _ in range(3):
        try:
            res = run_bass_kernel_spmd(nc, in_maps, list(range(8))).results
            break
        except Exception as e:  # noqa: BLE001
            last_err = e
    else:
        raise last_err
    yout = np.empty((NB, L, D), np.float32)
    for c in range(8):
        n, qi = divmod(c, 4)
        yout[n, qi * QT:(qi + 1) * QT] = res[c]["y"]
    return yout


if __name__ == "__main__":
    rng = np.random.default_rng(0)
    demo = {
        "x": rng.standard_normal((NB, L, D)).astype(np.float32),
        "w_qkv": rng.standard_normal((D, 3 * D)).astype(np.float32) * 0.03,
        "b_qkv": rng.standard_normal(3 * D).astype(np.float32) * 0.03,
        "w_o": rng.standard_normal((D, D)).astype(np.float32) * 0.03,
        "b_o": rng.standard_normal(D).astype(np.float32) * 0.03,
        "g1": np.ones(D, np.float32), "be1": np.zeros(D, np.float32),
        "w1": rng.standard_normal((D, FF)).astype(np.float32) * 0.03,
        "b1": rng.standard_normal(FF).astype(np.float32) * 0.03,
        "w2": rng.standard_normal((FF, D)).astype(np.float32) * 0.015,
        "b2": rng.standard_normal(D).astype(np.float32) * 0.015,
        "g2": np.ones(D, np.float32), "be2": np.zeros(D, np.float32),
    }
    out = kernel(**demo)
    print("kernel output:", out.shape, out.dtype, np.abs(out).mean())


# revision 21
# speedup vs baseline: 2.0318x; 1.0015x over previous
"""Trainium2 Bass kernel for a transformer encoder layer (nn_Encoder).

x:[2,2048,1024] f32. 8 NeuronCores, data-parallel: core c handles batch
n=c//4, query rows qi=c%4 (512 tokens). K/V are recomputed per core for
the full batch (x4 redundancy) to avoid collectives (~300us for the
8.4MB all-reduce this would replace).

All matmuls are fp8 e4m3 DoubleRow (2x PE column rate, 256-deep
contraction per instruction). K/Q are produced in a split-hd layout
([32 partitions, 2 k-tiles] per head) so even the hd=64 score matmuls
run DoubleRow. Weights are host-scaled x16 into the fp8 normal range;
the scale unwinds via the softmax ones-row (=16), the exp scale
(0.125/256) and a /256 on the fc2 PSUM. Residuals/LayerNorm stay f32.

The softmax exp stream on the Activation engine (~133us) is the
critical resource. Attention is split into two query-halves: while
half 1's exps run, half 0's output-proj/LN1/FFN execute in the PE/DVE
shadow (emitted as interleaved filler chunks); K/V projections for
later head-quads fill the shadow of half 0.
"""
import os
import sys
from contextlib import ExitStack

for _p in ("/opt/trn_rl_repo", "/root/.axon_site/_ro/trn_rl_repo"):
    if os.path.isdir(_p) and _p not in sys.path:
        sys.path.insert(0, _p)

import numpy as np
import concourse.bass as bass
import concourse.mybir as mybir
import concourse.tile as tile
from concourse import bacc
from concourse.bass_utils import run_bass_kernel_spmd
from concourse.masks import make_identity

F32 = mybir.dt.float32
F8 = mybir.dt.float8e4
AF = mybir.ActivationFunctionType
ALU = mybir.AluOpType
DR = mybir.MatmulPerfMode.DoubleRow

D = 1024
H = 16
HD = 64
FF = 4096
L = 2048
NB = 2
P = 128
QT = 512          # query tokens per core
QH = QT // 2      # query half
DC = D // P       # 8 chunks of the model dim
DC2 = DC // 2     # 4 DoubleRow chunk-pairs
KT = L // P       # 16 key tiles
FC = FF // P      # 32 ff chunks
FC2 = FC // 2     # 16 ff chunk-pairs
TT = QT // P      # 4 own-token tiles
NPAIR = H // 2    # 8 head pairs
NP2 = NPAIR // 2  # 4 pair-pairs
NQ = H // 4       # 4 head quads (scores split layout)
EPS = 1e-5
WS = 16.0         # host weight scale (fp8 range)

_CACHED_NC = {}


def _build_nc(affine=True):
    nc = bacc.Bacc("TRN2", target_bir_lowering=False)

    def dparam(name, shape, dt=F8):
        return nc.dram_tensor(name, shape, dt, kind="ExternalInput")

    xT8 = dparam("xT8", [D, L])            # x[n].T, fp8
    xTq8 = dparam("xTq8", [D, QT])         # own-token columns of xT, fp8
    xq = dparam("xq", [QT, D], F32)        # own tokens, natural (residual)
    # weights: partition-major fp8, DoubleRow k-tile-pair layouts
    wqs = dparam("wqs", [P, 2 * NQ, 2 * DC2 * P])    # [p][(g i)][d2 ik m]
    wks = dparam("wks", [P, 2 * NQ, 2 * DC2 * P])
    wv8 = dparam("wv8", [P, 2 * DC2 * 2 * 512])      # [p][vh d2 ik n]
    wo8 = dparam("wo8", [P, NP2 * 2 * D])            # [p][j ik o]
    w18 = dparam("w18", [P, FC // 4, 4 * (DC2 + 1) * 2 * P])  # [p][f4][f d2 ik m]
    w28 = dparam("w28", [P, FC2 // 2, 2 * 2 * D])        # [p][s][g ik o]
    bqs = dparam("bqs", [P, 2 * NQ], F32)  # x16 biases, split-hd order
    bks = dparam("bks", [P, 2 * NQ], F32)
    b1 = dparam("b1", [P, FC], F32)
    bvb = dparam("bvb", [P, D], F32)
    b2b = dparam("b2b", [P, D], F32)       # natural scale
    g1b = dparam("g1b", [P, D], F32)
    be1b = dparam("be1b", [P, D], F32)
    g2b = dparam("g2b", [P, D], F32)
    be2b = dparam("be2b", [P, D], F32)
    vones = dparam("vones", [P, KT])       # value 16 (denominator row)
    hc8 = dparam("hc8", [P, 2 * QT])       # F1 bias rows: [16,0...;0...]

    y = nc.dram_tensor("y", [QT, D], F32, kind="ExternalOutput")

    with tile.TileContext(nc) as tc, ExitStack() as es:
        # Pre-load the one activation table that serves every ACT func we
        # use (Exp for softmax, Ln+Exp for the LN rsqrt): without this the
        # first-fit chooser thrashes exp<->ln tables at 1283ns per load.
        from concourse.hw_specs import get_activation_tables
        _tabs = get_activation_tables(nc.m.arch)
        _idx = next(i for i, (_, s) in enumerate(_tabs.items())
                    if AF.Exp in s and AF.Ln in s)
        nc.scalar.add_instruction(mybir.InstLoadActFuncSet(
            name=nc.scalar.bass.get_next_instruction_name(),
            act_func_set_id=_idx, ins=[], outs=[]))

        pers = es.enter_context(tc.tile_pool(name="pers", bufs=1))
        ident = pers.tile([P, P], F32, tag="ident")
        make_identity(nc, ident[:])
        bqs_t = pers.tile([P, 2 * NQ], F32, tag="bqs")
        bks_t = pers.tile([P, 2 * NQ], F32, tag="bks")
        b1_t = pers.tile([P, FC], F32, tag="b1")
        bvb_t = pers.tile([P, D], F32, tag="bvb")
        # b2b reuses bvb's slot: bvb is dead after V-proj, long before
        # the LNT chunks fold b2 into the residual.
        b2b_t = pers.tile([P, D], F32, tag="bvb", name="b2b_t")

        xT_t = pers.tile([P, DC, L], F8, tag="xT")
        xTq_t = pers.tile([P, DC, QT], F8, tag="xTq")
        v_aug = pers.tile([P, KT, H * (HD + 1)], F8, tag="vaug")
        ones_t = pers.tile([P, KT], F8, tag="ones")
        qT_all = pers.tile([P, NQ, 2, QT], F8, tag="qT")
        kT_all = pers.tile([P, NQ, 2, L], F8, tag="kT")
        outSB = pers.tile([P, NPAIR, QT], F8, tag="outSB")
        h_t = pers.tile([P, TT, D], F32, tag="h")
        hT_t = pers.tile([P, DC + 2, QT], F8, tag="hT")
        ff1 = pers.tile([P, FC, QT], F8, tag="ff1")
        xq_s = pers.tile([P, TT, D], F32, tag="xqs")
        wqs_t = pers.tile([P, 2 * NQ, DC2, 2 * P], F8, tag="wqs")
        wks_t = pers.tile([P, 2 * NQ, DC2, 2 * P], F8, tag="wks")
        wv_t = pers.tile([P, 2, DC2, 2 * 512], F8, tag="wv")
        wo_t = pers.tile([P, NP2, 2 * D], F8, tag="wof")
        w2_t = pers.tile([P, FC2 // 2, 2, 2 * D], F8, tag="w2")
        if affine:
            g1b_t = pers.tile([P, D], F32, tag="g1b")
            be1b_t = pers.tile([P, D], F32, tag="be1b")
            g2b_t = pers.tile([P, D], F32, tag="g2b")
            be2b_t = pers.tile([P, D], F32, tag="be2b")
        else:
            g1b_t = be1b_t = g2b_t = be2b_t = None

        # startup DMAs, ordered for earliest first-exp: the DMA engine pool
        # is serially occupied, so issue exactly what unblocks Q/K/V first.
        nc.scalar.dma_start(bqs_t[:], bqs[:])
        nc.scalar.dma_start(bks_t[:], bks[:])
        nc.sync.dma_start(xTq_t[:], xTq8.rearrange("(c p) t -> p c t", p=P))
        nc.sync.dma_start(
            wqs_t[:], wqs.rearrange("p b (c m) -> p b c m", c=DC2))
        nc.sync.dma_start(
            wks_t[:], wks.rearrange("p b (c m) -> p b c m", c=DC2))
        for blk in range(4):
            nc.sync.dma_start(
                xT_t[:, :, blk * 512:(blk + 1) * 512],
                xT8.rearrange("(c p) t -> p c t", p=P)[
                    :, :, blk * 512:(blk + 1) * 512])
        nc.sync.dma_start(
            wv_t[:], wv8.rearrange("p (v c m) -> p v c m", v=2, c=DC2))
        nc.scalar.dma_start(b1_t[:], b1[:])
        nc.scalar.dma_start(bvb_t[:], bvb[:])
        nc.scalar.dma_start(ones_t[:], vones[:])
        nc.scalar.dma_start(
            hT_t[:, DC:DC + 2, :],
            hc8.rearrange("p (k t) -> p k t", k=2))

        nc.vector.tensor_copy(
            v_aug[:].rearrange("p t (h c) -> p t h c", c=HD + 1)[:, :, :, HD],
            ones_t[:, :, None].to_broadcast([P, KT, H]))
        # mid-kernel loads, all needed only by the post-attention chunks
        nc.sync.dma_start(xq_s[:], xq.rearrange("(t p) d -> p t d", p=P))
        nc.sync.dma_start(wo_t[:], wo8.rearrange("p (j m) -> p j m", j=NP2))
        nc.sync.dma_start(w2_t[:], w28.rearrange("p s (g m) -> p s g m", g=2))
        if affine:
            nc.scalar.dma_start(g1b_t[:], g1b[:])
            nc.scalar.dma_start(be1b_t[:], be1b[:])
            nc.scalar.dma_start(g2b_t[:], g2b[:])
            nc.scalar.dma_start(be2b_t[:], be2b[:])

        stp = es.enter_context(tc.tile_pool(name="stp", bufs=2, space="PSUM"))
        pvp = es.enter_context(tc.tile_pool(name="pvp", bufs=2, space="PSUM"))
        ppp = es.enter_context(tc.tile_pool(name="ppp", bufs=2))
        atd = es.enter_context(tc.tile_pool(name="atd", bufs=1))
        lnp = es.enter_context(tc.tile_pool(name="lnp", bufs=2))
        fp = es.enter_context(tc.tile_pool(name="fp", bufs=2))
        ft = es.enter_context(tc.tile_pool(name="ft", bufs=2))
        kvp_es = ExitStack()
        kvp = kvp_es.enter_context(tc.tile_pool(name="kvp", bufs=2,
                                                space="PSUM"))

        def emit_qproj(g):
            for i in range(2):
                b = 2 * g + i
                ps = kvp.tile([P, 512], F32, tag="kv", name=f"qps_{b}")
                for d2 in range(DC2):
                    nc.tensor.matmul(
                        ps[:],
                        wqs_t[:, b, d2, :].rearrange("p (i m) -> p i m", i=2),
                        xTq_t[:, 2 * d2:2 * d2 + 2, :],
                        start=(d2 == 0), stop=(d2 == DC2 - 1), perf_mode=DR)
                nc.vector.tensor_scalar(
                    qT_all[:, g, i, :], ps[:],
                    bqs_t[:, b:b + 1], None, ALU.add)

        def emit_kproj(g, i):
            """kT_all[:, g, i, :] for one hd-half of head-quad g."""
            b = 2 * g + i
            for blk in range(4):     # 512-key blocks
                ps = kvp.tile([P, 512], F32, tag="kv",
                              name=f"kps_{g}_{i}_{blk}")
                for d2 in range(DC2):
                    nc.tensor.matmul(
                        ps[:],
                        wks_t[:, b, d2, :].rearrange("p (i m) -> p i m", i=2),
                        xT_t[:, 2 * d2:2 * d2 + 2,
                             blk * 512:(blk + 1) * 512],
                        start=(d2 == 0), stop=(d2 == DC2 - 1), perf_mode=DR)
                nc.vector.tensor_scalar(
                    kT_all[:, g, i, blk * 512:(blk + 1) * 512], ps[:],
                    bks_t[:, b:b + 1], None, ALU.add)

        def emit_vsub(vh, sub):
            """v_aug columns for v-half vh, key tiles 4*sub..4*sub+3."""
            for tt in range(4 * sub, 4 * sub + 4):
                ps = kvp.tile([P, 512], F32, tag="kv",
                              name=f"vps_{vh}_{tt}")
                for d2 in range(DC2):
                    nc.tensor.matmul(
                        ps[:],
                        xT_t[:, 2 * d2:2 * d2 + 2, tt * P:(tt + 1) * P],
                        wv_t[:, vh, d2, :].rearrange("p (i n) -> p i n", i=2),
                        start=(d2 == 0), stop=(d2 == DC2 - 1), perf_mode=DR)
                dst = v_aug[:, tt, :].rearrange(
                    "p (h c) -> p h c", c=HD + 1)[:, vh * 8:(vh + 1) * 8, 0:HD]
                nc.vector.tensor_tensor(
                    dst, ps[:].rearrange("p (h c) -> p h c", c=HD),
                    bvb_t[:, vh * 512:(vh + 1) * 512].rearrange(
                        "p (h c) -> p h c", c=HD),
                    ALU.add)

        emit_qproj(0)
        emit_kproj(0, 0)
        emit_kproj(0, 1)
        emit_vsub(0, 0)

        fillers = []

        def drain():
            if fillers:
                fillers.pop(0)()

        def emit_attn(qh, pr):
            g, j0 = pr // 2, 2 * (pr % 2)
            pvs = [pvp.tile([P, 512], F32, tag="pv", name=f"pv_{qh}_{pr}_{h2}")
                   for h2 in range(2)]
            for grp in range(KT // 2):
                st = stp.tile([P, 2, 2, 256], F32, tag="st",
                              name=f"st_{qh}_{pr}_{grp}")
                for h2 in range(2):
                    j = j0 + h2
                    rows = slice(32 * j, 32 * j + 32)
                    for k in range(2):
                        kt = 2 * grp + k
                        nc.tensor.matmul(
                            st[:, h2, k, :],
                            kT_all[rows, g, :, kt * P:(kt + 1) * P],
                            qT_all[rows, g, :, qh * QH:(qh + 1) * QH],
                            start=True, stop=True, perf_mode=DR,
                            tile_position=(32 * j, 0))
                pp = ppp.tile([P, 2, 2, 256], F8, tag="pp",
                              name=f"pp_{qh}_{pr}_{grp}")
                nc.scalar.activation(pp[:], st[:], AF.Exp,
                                     scale=0.125 / (WS * WS))
                for h2 in range(2):
                    h = 2 * pr + h2
                    nc.tensor.matmul(
                        pvs[h2][:HD + 1, :QH],
                        v_aug[:, 2 * grp:2 * grp + 2,
                              h * (HD + 1):(h + 1) * (HD + 1)],
                        pp[:, h2, :, :],
                        start=(grp == 0), stop=(grp == KT // 2 - 1),
                        perf_mode=DR)
                if grp in (0, 2, 5):
                    drain()
            for h2 in range(2):
                rows = slice(h2 * HD, h2 * HD + HD)
                # bounce PV to SBUF so the PSUM bank frees after one short
                # DVE copy instead of the whole recip/bcast/normalize chain
                # (the bank gates the next pair's PV accumulator).
                pvc = atd.tile([P, QH], F32, tag="pvc",
                               name=f"pvc_{qh}_{pr}_{h2}")
                nc.vector.tensor_copy(pvc[:HD + 1, :], pvs[h2][:HD + 1, :QH])
                den = atd.tile([1, QH], F32, tag="den",
                               name=f"den_{qh}_{pr}_{h2}")
                nc.vector.reciprocal(den[:], pvc[HD:HD + 1, :])
                denb = atd.tile([HD, QH], F32, tag="denb",
                                name=f"denb_{qh}_{pr}_{h2}")
                nc.gpsimd.partition_broadcast(denb[:], den[:])
                nc.gpsimd.tensor_tensor(
                    outSB[rows, pr, qh * QH:(qh + 1) * QH],
                    pvc[:HD, :], denb[:], ALU.mult)

        # ---------- post-attention chunk emitters (token-tile tg) ----------
        postp_es = ExitStack()
        postp = [None]

        def emit_O(qh, tt):
            tg = 2 * qh + tt
            for oc in range(2):
                ps = postp[0].tile([P, 512], F32, tag="post",
                                   name=f"ops_{tg}_{oc}")
                for j2 in range(NP2):
                    nc.tensor.matmul(
                        ps[:],
                        outSB[:, 2 * j2:2 * j2 + 2, tg * P:(tg + 1) * P],
                        wo_t[:, j2, :].rearrange(
                            "p (i o) -> p i o", i=2)[:, :, oc * 512:(oc + 1) * 512],
                        start=(j2 == 0), stop=(j2 == NP2 - 1), perf_mode=DR)
                nc.vector.tensor_tensor(
                    h_t[:, tg, oc * 512:(oc + 1) * 512], ps[:],
                    xq_s[:, tg, oc * 512:(oc + 1) * 512], ALU.add)

        def emit_LNT(qh, tt):
            tg = 2 * qh + tt
            _layernorm(nc, lnp, h_t[:, tg, :], h_t[:, tg, :],
                       g1b_t, be1b_t, affine)
            for dcg in range(2):
                tp = postp[0].tile([P, 512], F32, tag="post",
                                   name=f"tp_{tg}_{dcg}")
                for k in range(4):
                    dc = 4 * dcg + k
                    nc.tensor.transpose(
                        tp[:, k * P:(k + 1) * P],
                        h_t[:, tg, dc * P:(dc + 1) * P], ident[:])
                nc.vector.tensor_copy(
                    hT_t[:, 4 * dcg:4 * dcg + 4, tg * P:(tg + 1) * P],
                    tp[:].rearrange("p (k m) -> p k m", k=4))
            # fold the fc2 bias into the residual (after transposes read h)
            nc.vector.tensor_tensor(h_t[:, tg, :], h_t[:, tg, :],
                                    b2b_t[:], ALU.add)

        def emit_F1(qh, fcg, use_act=False):
            w1_t = fp.tile([P, 4, DC2 + 1, 2 * P], F8, tag="w1s")
            nc.sync.dma_start(w1_t[:], w18[:, fcg, :].rearrange(
                "p (f c m) -> p f c m", f=4, c=DC2 + 1))
            for u in range(2):
                ps = postp[0].tile([P, 512], F32, tag="post",
                                   name=f"f1_{qh}_{fcg}_{u}")
                for f in (2 * u, 2 * u + 1):
                    fc = 4 * fcg + f
                    for d2 in range(DC2 + 1):
                        nc.tensor.matmul(
                            ps[:, (f - 2 * u) * QH:(f - 2 * u + 1) * QH],
                            w1_t[:, f, d2, :].rearrange(
                                "p (i m) -> p i m", i=2),
                            hT_t[:, 2 * d2:2 * d2 + 2, qh * QH:(qh + 1) * QH],
                            start=(d2 == 0), stop=(d2 == DC2), perf_mode=DR,
                            skip_group_check=True)
                dst = ff1[:, 4 * fcg + 2 * u:4 * fcg + 2 * u + 2,
                          qh * QH:(qh + 1) * QH]
                src_ap = ps[:].rearrange("p (f n) -> p f n", f=2)
                if use_act:
                    nc.scalar.activation(dst, src_ap, AF.Relu)
                else:
                    nc.vector.tensor_scalar(dst, src_ap, 0.0, None, ALU.max)

        def emit_F2(qh, tt, oc, part=2):
            """part: 0 = first half of the ff contraction, 1 = second half
            (+LN2/store), 2 = whole thing in one chunk."""
            tg = 2 * qh + tt
            if oc == 0 and part != 1:
                _f2_t2[tg] = ft.tile([P, D], F32, tag="t2", name=f"t2_{tg}")
            lo = FC2 // 2 if part == 1 else 0
            hi = FC2 // 2 if part == 0 else FC2
            ps = postp[0].tile([P, 512], F32, tag="post",
                               name=f"f2_{tg}_{oc}_{part}")
            for f2 in range(lo, hi):
                s, g2 = f2 // 2, f2 % 2
                nc.tensor.matmul(
                    ps[:],
                    ff1[:, 2 * f2:2 * f2 + 2, tg * P:(tg + 1) * P],
                    w2_t[:, s, g2, :].rearrange(
                        "p (i o) -> p i o", i=2)[:, :, oc * 512:(oc + 1) * 512],
                    start=(f2 == lo), stop=(f2 == hi - 1), perf_mode=DR)
            t2 = _f2_t2[tg]
            acc = h_t[:, tg, oc * 512:(oc + 1) * 512] if part != 1 \
                else t2[:, oc * 512:(oc + 1) * 512]
            nc.vector.scalar_tensor_tensor(
                t2[:, oc * 512:(oc + 1) * 512], ps[:], 1.0 / (WS * WS),
                acc, ALU.mult, ALU.add)
            if oc == 1 and part != 0:
                _layernorm(nc, lnp, t2[:], t2[:], g2b_t, be2b_t, affine)
                nc.sync.dma_start(
                    y.rearrange("(t p) d -> p t d", p=P)[:, tg, :], t2[:])

        _f2_t2 = {}

        # ---------------- schedule ----------------
        fillers.extend([
            lambda: emit_vsub(0, 1), lambda: emit_vsub(0, 2),
            lambda: emit_vsub(0, 3),
            lambda: emit_qproj(1),
            lambda: emit_kproj(1, 0), lambda: emit_kproj(1, 1),
            lambda: emit_vsub(1, 0), lambda: emit_vsub(1, 1),
            lambda: emit_qproj(2),
            lambda: emit_kproj(2, 0), lambda: emit_kproj(2, 1),
            lambda: emit_vsub(1, 2), lambda: emit_vsub(1, 3),
            lambda: emit_qproj(3),
            lambda: emit_kproj(3, 0), lambda: emit_kproj(3, 1),
        ])
        for pr in range(NPAIR):
            emit_attn(0, pr)
        while fillers:
            drain()
        kvp_es.close()
        nc.scalar.dma_start(b2b_t[:], b2b[:])
        postp[0] = postp_es.enter_context(
            tc.tile_pool(name="postp", bufs=2, space="PSUM"))
        fillers.extend(
            [lambda tt=tt: emit_O(0, tt) for tt in range(2)] +
            [lambda tt=tt: emit_LNT(0, tt) for tt in range(2)] +
            [lambda f=f: emit_F1(0, f) for f in range(FC // 4)] +
            [lambda tt=tt, oc=oc: emit_F2(0, tt, oc)
             for tt in range(2) for oc in range(2)])
        for pr in range(NPAIR):
            emit_attn(1, pr)
        while fillers:
            drain()
        for tt in range(2):
            emit_O(1, tt)
            emit_LNT(1, tt)
        for f in range(FC // 4):
            emit_F1(1, f, use_act=True)
            if f == 4:
                for tt in range(2):
                    for oc in range(2):
                        emit_F2(1, tt, oc, part=0)
        for tt in range(2):
            for oc in range(2):
                emit_F2(1, tt, oc, part=1)
        postp_es.close()

    nc.compile()
    return nc


def _layernorm(nc, pool, dst, src, g_t, be_t, affine):
    """dst = (src - mean)/sqrt(var + eps) [* g + be], row-wise over 1024.

    bn_stats/bn_aggr produce mean+var in one DVE pass. rsqrt is computed
    as exp(-0.5*ln(v)) on ACT (both funcs live in one activation table,
    so no table thrash with the attention exps) and refined with one
    Newton step on DVE.
    """
    stats = pool.tile([P, 2, 6], F32, tag="ln_st")
    nc.vector.bn_stats(stats[:, 0, :], src[:, 0:D // 2])
    nc.vector.bn_stats(stats[:, 1, :], src[:, D // 2:D])
    mv = pool.tile([P, 2], F32, tag="ln_mv")
    nc.vector.bn_aggr(mv[:], stats[:])
    vv = pool.tile([P, 1], F32, tag="ln_v")
    nc.vector.tensor_scalar(vv[:], mv[:, 1:2], EPS, None, ALU.add)
    lnv = pool.tile([P, 1], F32, tag="ln_ln")
    nc.scalar.activation(lnv[:], vv[:], AF.Ln)
    r = pool.tile([P, 1], F32, tag="ln_r")
    nc.scalar.activation(r[:], lnv[:], AF.Exp, scale=-0.5)
    # one Newton step: r <- r * (1.5 - 0.5 * vv * r^2)
    t = pool.tile([P, 1], F32, tag="ln_t")
    nc.vector.tensor_tensor(t[:], r[:], r[:], ALU.mult)
    nc.vector.tensor_tensor(t[:], t[:], vv[:], ALU.mult)
    nc.vector.tensor_scalar(t[:], t[:], -0.5, 1.5, ALU.mult, ALU.add)
    nc.vector.tensor_tensor(r[:], r[:], t[:], ALU.mult)
    nc.vector.tensor_scalar(dst, src, mv[:, 0:1], r[:], ALU.subtract, ALU.mult)
    if affine:
        nc.vector.tensor_tensor(dst, dst, g_t[:], ALU.mult)
        nc.vector.tensor_tensor(dst, dst, be_t[:], ALU.add)


def _hc8():
    h = np.zeros((P, 2 * QT), np.float32)
    h[0, :QT] = WS
    return h.astype(mybir.dt.np(F8))


def make_in_maps(x, w_qkv, b_qkv, w_o, b_o, g1, be1, w1, b1, w2, b2, g2, be2):
    f = np.float32
    f8 = mybir.dt.np(F8)
    x = np.asarray(x, f)
    w_qkv = np.asarray(w_qkv, f)
    b_qkv = np.asarray(b_qkv, f)
    bc = lambda v: np.ascontiguousarray(
        np.broadcast_to(np.asarray(v, f).reshape(1, D), (P, D)))

    # [d, h*64+hd] -> [p][(g i)][d2 ik m] with m=32j+r -> (head 4g+j, hd 32i+r)
    def qk_split(w):
        t = (w * WS).reshape(DC2, 2, P, NQ, 4, 2, 32)
        t = t.transpose(2, 3, 5, 0, 1, 4, 6)    # [p, g, i, d2, ik, j, r]
        return np.ascontiguousarray(t.reshape(P, 2 * NQ, 2 * DC2 * P)).astype(f8)

    def bias_split(b):
        t = (b * WS).reshape(NQ, 4, 2, 32).transpose(1, 3, 0, 2)  # [j, r, g, i]
        return np.ascontiguousarray(t.reshape(P, 2 * NQ))

    wv_h = np.ascontiguousarray(
        (w_qkv[:, 2 * D:] * WS).reshape(DC2, 2, P, 2, 512)
        .transpose(2, 3, 0, 1, 4).reshape(P, 2 * DC2 * 2 * 512)).astype(f8)
    wo_h = np.ascontiguousarray(
        np.asarray(w_o, f).reshape(NP2, 2, P, D).transpose(2, 0, 1, 3)
        .reshape(P, NP2 * 2 * D)).astype(f8)
    w1_base = ((np.asarray(w1, f) * WS).reshape(DC2, 2, P, FC // 4, 4, P)
               .transpose(2, 3, 4, 0, 1, 5))          # [p, f4, f, d2, ik, m]
    w1_bias = np.zeros((P, FC // 4, 4, 1, 2, P), f)
    w1_bias[0, :, :, 0, 0, :] = np.asarray(b1, f).reshape(FC // 4, 4, P)
    w1_h = np.ascontiguousarray(
        np.concatenate([w1_base, w1_bias], axis=3)
        .reshape(P, FC // 4, 4 * (DC2 + 1) * 2 * P)).astype(f8)
    w2_h = np.ascontiguousarray(
        (np.asarray(w2, f) * WS).reshape(FC2 // 2, 2, 2, P, D)
        .transpose(3, 0, 1, 2, 4).reshape(P, FC2 // 2, 2 * 2 * D)).astype(f8)

    shared = {
        "wqs": qk_split(w_qkv[:, :D]),
        "wks": qk_split(w_qkv[:, D:2 * D]),
        "wv8": wv_h, "wo8": wo_h, "w18": w1_h, "w28": w2_h,
        "bqs": bias_split(b_qkv[:D]),
        "bks": bias_split(b_qkv[D:2 * D]),
        "b1": np.ascontiguousarray((np.asarray(b1, f) * WS).reshape(FC, P).T),
        "bvb": bc(np.asarray(b_qkv[2 * D:], f) * WS), "b2b": bc(b2),
        "g1b": bc(g1), "be1b": bc(be1), "g2b": bc(g2), "be2b": bc(be2),
        "vones": np.full((P, KT), WS, f).astype(f8),
        "hc8": _hc8(),
    }
    in_maps = []
    for c in range(8):
        n, qi = divmod(c, 4)
        xT8n = np.ascontiguousarray(x[n].T).astype(f8)
        m = dict(shared)
        m["xT8"] = xT8n
        m["xTq8"] = np.ascontiguousarray(xT8n[:, qi * QT:(qi + 1) * QT])
        m["xq"] = np.ascontiguousarray(x[n, qi * QT:(qi + 1) * QT, :]
                                       + np.asarray(b_o, f).reshape(1, D))
        in_maps.append(m)
    return in_maps


def get_nc(affine=True):
    if affine not in _CACHED_NC:
        _CACHED_NC[affine] = _build_nc(affine)
    return _CACHED_NC[affine]


def kernel(**inputs):
    in_maps = make_in_maps(**inputs)
    affine = not (np.all(np.asarray(inputs["g1"]) == 1)
                  and np.all(np.asarray(inputs["be1"]) == 0)
                  and np.all(np.asarray(inputs["g2"]) == 1)
                  and np.all(np.asarray(inputs["be2"]) == 0))
    nc = get_nc(affine)
    # The axon-proxied NRT occasionally reports a transient
    # NRT_EXEC_UNIT_UNRECOVERABLE on a cold first dispatch; a plain retry
    # has always succeeded with bit-identical results, so recover inline.
    last_err = None
    for _ in range(3):
        try:
            res = run_bass_kernel_spmd(nc, in_maps, list(range(8))).results
            break
        except Exception as e:  # noqa: BLE001
            last_err = e
    else:
        raise last_err
    yout = np.empty((NB, L, D), np.float32)
    for c in range(8):
        n, qi = divmod(c, 4)
        yout[n, qi * QT:(qi + 1) * QT] = res[c]["y"]
    return yout


if __name__ == "__main__":
    rng = np.random.default_rng(0)
    demo = {
        "x": rng.standard_normal((NB, L, D)).astype(np.float32),
        "w_qkv": rng.standard_normal((D, 3 * D)).astype(np.float32) * 0.03,
        "b_qkv": rng.standard_normal(3 * D).astype(np.float32) * 0.03,
        "w_o": rng.standard_normal((D, D)).astype(np.float32) * 0.03,
        "b_o": rng.standard_normal(D).astype(np.float32) * 0.03,
        "g1": np.ones(D, np.float32), "be1": np.zeros(D, np.float32),
        "w1": rng.standard_normal((D, FF)).astype(np.float32) * 0.03,
        "b1": rng.standard_normal(FF).astype(np.float32) * 0.03,
        "w2": rng.standard_normal((FF, D)).astype(np.float32) * 0.015,
        "b2": rng.standard_normal(D).astype(np.float32) * 0.015,
        "g2": np.ones(D, np.float32), "be2": np.zeros(D, np.float32),
    }
    out = kernel(**demo)
    print("kernel output:", out.shape, out.dtype, np.abs(out).mean())
